# revision 32
# baseline (speedup 1.0000x reference)
"""Trainium2 Bass kernel for BriaFibo single transformer block.

Tensor-parallel over 8 NeuronCores: heads (24 -> 3/core) and mlp_hidden
(12288 -> 1536/core) are column-sharded; out projection row-sharded with a
device-side ReduceScatter.  AdaLN emb matvec is row-sharded + AllGather.
All big matmuls run in float32r (full PE rate at N>=256, ~fp32 storage).
"""

import ml_dtypes
import numpy as np

import concourse.bass as bass
import concourse.mybir as mybir
import concourse.tile as tile
from concourse import bacc
from concourse.bass_utils import run_bass_kernel_spmd

F32 = mybir.dt.float32
F32R = mybir.dt.float32r
BF16 = mybir.dt.bfloat16
AOP = mybir.AluOpType
AF = mybir.ActivationFunctionType

S, D = 2048, 3072
HEADS, HD = 24, 128
MH = 12288
NCORES = 8
HPC = HEADS // NCORES          # 3 heads/core
QKV = HPC * HD                 # 384
MHC = MH // NCORES             # 1536
CAT = QKV + MHC                # 1920
SO = S // NCORES               # 256 output rows/core
KT = D // 128                  # 24 contraction tiles
EMBC = 3 * D // NCORES         # 1152 adaLN rows/core
EPS_LN = 1e-6
EPS_RMS = 1e-6

TRACE = False
RSCHUNKS = 6
TIME_ITERS = 0
DEBUG = False
SIM = False
LAST = {}


def _r(ap):
    return ap.bitcast(F32R)



def _build():
    nc = bacc.Bacc("TRN2", target_bir_lowering=False, debug=False,
                   num_devices=NCORES)

    din = {}
    for name, shape, dt in [
        ("hs", [S, D], F32), ("hs_res", [SO, D], F32), ("temb", [D], F32),
        ("cosT", [HD, S], F32), ("sinT", [HD, S], F32),
        ("qkvwT", [D, 3 * QKV], BF16), ("qkvb", [3 * QKV], F32),
        ("mlpwT", [D, MHC], BF16), ("mlpb", [MHC], F32),
        ("outwT", [CAT, D], BF16), ("outb", [D], F32),
        ("nwT", [D, EMBC], BF16), ("nb", [EMBC], F32),
        ("rmsq", [HD], F32), ("rmsk", [HD], F32), ("ident", [128, 128], F32),
    ]:
        din[name] = nc.dram_tensor(name, shape, dt, kind="ExternalInput")
    out_d = nc.dram_tensor("out", [SO, D], F32, kind="ExternalOutput")
    dbg = {}
    if DEBUG:
        for name, shape in [("demb", [3 * D]), ("dnh", [KT, 128, 256]),
                            ("dqk", [2 * HPC, 128, 256]),
                            ("dv", [128, S // 128, QKV]),
                            ("dattn", [128, HPC, S]),
                            ("dhid", [128, 12, 256])]:
            dbg[name] = nc.dram_tensor(name, shape, F32,
                                       kind="ExternalOutput")

    from contextlib import ExitStack
    with tile.TileContext(nc) as tc, ExitStack() as ctx:
        _emit(ctx, nc, tc, din, out_d, dbg)
    nc.compile()
    return nc


def _emit(ctx, nc, tc, din, out_d, dbg=None):
    hs, hs_res = din["hs"], din["hs_res"]

    cpool = ctx.enter_context(tc.tile_pool(name="consts", bufs=1))
    dram = ctx.enter_context(tc.tile_pool(name="dram", bufs=1, space="DRAM"))

    ident_sb = cpool.tile([128, 128], F32)
    nc.sync.dma_start(out=ident_sb[:], in_=din["ident"][:, :])
    ident_bf = cpool.tile([128, 128], BF16)
    nc.gpsimd.dma_start(out=ident_bf[:], in_=din["ident"][:, :])
    ones_f = cpool.tile([128, 128], F32)
    nc.vector.memset(ones_f[:], 1.0)
    ones_col = cpool.tile([128, 1], F32R)         # lhsT for colsum -> [1,N]
    nc.vector.tensor_copy(ones_col[:], ones_f[:, 0:1])
    ones_col_bf = cpool.tile([128, 1], BF16)      # bf16 colsum lhsT
    nc.vector.tensor_copy(ones_col_bf[:], ones_f[:, 0:1])
    ones_row = cpool.tile([1, 128], F32)          # lhsT for bcast -> [128,N]
    nc.vector.tensor_copy(ones_row[:], ones_f[0:1, :])
    eps_ln_c = cpool.tile([128, 1], F32)
    nc.vector.memset(eps_ln_c[:], EPS_LN)
    eps_rms_c = cpool.tile([1, 1], F32)
    nc.vector.memset(eps_rms_c[:], EPS_RMS)

    rmsq_col = cpool.tile([128, 1], F32)
    nc.gpsimd.dma_start(out=rmsq_col[:],
                        in_=din["rmsq"].rearrange("(p one) -> p one", one=1))
    rmsk_col = cpool.tile([128, 1], F32)
    nc.gpsimd.dma_start(out=rmsk_col[:],
                        in_=din["rmsk"].rearrange("(p one) -> p one", one=1))
    qkvb_cols = cpool.tile([128, 9], F32)
    nc.gpsimd.dma_start(out=qkvb_cols[:],
                        in_=din["qkvb"].rearrange("(m p) -> p m", p=128))
    vb_b = cpool.tile([128, QKV], F32)
    vb_src = din["qkvb"][768:1152]
    nc.gpsimd.dma_start(
        out=vb_b[:],
        in_=bass.AP(vb_src.tensor, vb_src.offset, [[0, 128], [1, QKV]]))
    mlpb_cols = cpool.tile([128, 12], F32)
    nc.gpsimd.dma_start(out=mlpb_cols[:],
                        in_=din["mlpb"].rearrange("(m p) -> p m", p=128))

    # DRAM scratch
    nhT_sp = dram.tile([KT, 128, S], BF16)
    qkT_sp = dram.tile([2 * HPC, 128, S], BF16)
    ag_in = dram.tile([EMBC], F32)
    rk_b = dram.tile([S], F32)
    emb_all = dram.tile([3 * D], F32, addr_space="Shared")
    NC6 = 6                                     # out-proj column chunks
    CW = D // NC6                               # 512 cols per chunk
    RSCH = globals().get("RSCHUNKS", RSCHUNKS)  # collectives count
    RW = D // RSCH
    GPC = NC6 // RSCH                           # compute chunks per RS
    partial_c = [dram.tile([S, RW], BF16, name=f"partial{i}")
                 for i in range(RSCH)]
    rs_c = [dram.tile([SO, RW], BF16, name=f"rsc{i}") for i in range(RSCH)]

    # ---------------- Phase 0: AdaLN emb (sharded matvec + AllGather) ----
    with tc.tile_pool(name="p0", bufs=1) as p0, \
         tc.tile_pool(name="p0st", bufs=3) as p0st, \
         tc.tile_pool(name="p0ps", bufs=1, space="PSUM") as p0ps:
        temb_sb = p0.tile([128, KT], F32)
        nc.gpsimd.dma_start(out=temb_sb[:],
                            in_=din["temb"].rearrange("(a p) -> p a", p=128))
        silu_t = p0.tile([128, KT], BF16)
        nc.scalar.activation(silu_t[:], temb_sb[:], AF.Silu)
        pe_all = p0ps.tile([1, 3, 512], F32)
        for k in range(KT):
            nw_k = p0st.tile([128, EMBC], BF16, name="nw_k")
            nc.sync.dma_start(out=nw_k[:],
                              in_=din["nwT"][k * 128:(k + 1) * 128, :])
            for n in range(3):
                nc.tensor.matmul(pe_all[:, n, 0:384],
                                 silu_t[:, k:k + 1],
                                 nw_k[:, n * 384:(n + 1) * 384],
                                 start=(k == 0), stop=(k == KT - 1))
        nb_sb = p0.tile([1, EMBC], F32)
        nc.sync.dma_start(out=nb_sb[:],
                          in_=din["nb"].rearrange("(one a) -> one a", one=1))
        emb_row = p0.tile([1, EMBC], F32)
        for n in range(3):
            nc.vector.tensor_add(emb_row[:, n * 384:(n + 1) * 384],
                                 pe_all[:, n, 0:384],
                                 nb_sb[:, n * 384:(n + 1) * 384])
        nc.sync.dma_start(out=ag_in[:], in_=emb_row[:])
        if SIM:
            nc.sync.dma_start(out=emb_all[0:EMBC], in_=ag_in[:])
        else:
            nc.gpsimd.collective_compute(
                "AllGather", AOP.bypass,
                replica_groups=[list(range(NCORES))],
                ins=[ag_in.opt()], outs=[emb_all.opt()])

    if dbg:
        nc.sync.dma_start(out=dbg["demb"][:], in_=emb_all[:])

    scale_cols = cpool.tile([128, KT], F32)
    sc_src = emb_all[D:2 * D]
    nc.gpsimd.dma_start(
        out=scale_cols[:],
        in_=bass.AP(sc_src.tensor, sc_src.offset, [[1, 128], [128, KT]]))
    nc.vector.tensor_scalar_add(scale_cols[:], scale_cols[:], 1.0)
    shift_cols = cpool.tile([128, KT], F32)
    sh_src = emb_all[0:D]
    nc.gpsimd.dma_start(
        out=shift_cols[:],
        in_=bass.AP(sh_src.tensor, sh_src.offset, [[1, 128], [128, KT]]))

    # V stays SBUF-resident (bf16) from phase 1 through attention
    vres = ctx.enter_context(tc.tile_pool(name="vres", bufs=1))
    v_sb2 = vres.tile([128, S // 128, QKV], BF16)

    # ---------------- Phase 1: LN + transpose + qkv/v projections --------
    # bf16 pipeline, 512-token blocks, resident bf16 qkv weights
    NB2, BT2 = 4, 512
    with tc.tile_pool(name="p1w", bufs=1) as p1w, \
         tc.tile_pool(name="p1hs", bufs=2) as p1hs, \
         tc.tile_pool(name="p1st", bufs=3) as p1st, \
         tc.tile_pool(name="p1nh", bufs=2) as p1nh, \
         tc.tile_pool(name="p1ev", bufs=3) as p1ev, \
         tc.tile_pool(name="p1ps", bufs=2, space="PSUM") as p1ps, \
         tc.tile_pool(name="p1psT", bufs=2, space="PSUM") as p1psT:
        qkvw_k = [p1w.tile([128, 3 * QKV], BF16, name=f"qw{k}")
                  for k in range(KT)]
        for k in range(KT):
            nc.sync.dma_start(out=qkvw_k[k][:],
                              in_=din["qkvwT"][k * 128:(k + 1) * 128, :])
        for b in range(NB2):
            nhT_b = p1nh.tile([128, KT, BT2], BF16, name="nhT_b")
            hts = []
            for tt in range(4):
                row = b * BT2 + tt * 128
                h0 = p1hs.tile([128, D // 2], BF16, name=f"h{tt}a",
                               tag=f"h{tt}a")
                nc.gpsimd.dma_start(out=h0[:],
                                    in_=hs[row:row + 128, 0:D // 2])
                h1 = p1hs.tile([128, D // 2], BF16, name=f"h{tt}b",
                               tag=f"h{tt}b")
                nc.gpsimd.dma_start(out=h1[:],
                                    in_=hs[row:row + 128, D // 2:D])
                stats = p1st.tile([128, 6, 6], F32, name="stats")
                for g in range(3):
                    nc.vector.bn_stats(stats[:, g, :],
                                       h0[:, g * 512:(g + 1) * 512])
                    nc.vector.bn_stats(stats[:, 3 + g, :],
                                       h1[:, g * 512:(g + 1) * 512])
                mv = p1st.tile([128, 2], F32, name="mv")
                nc.vector.bn_aggr(mv[:], stats[:])
                sd = p1st.tile([128, 1], F32, name="sd")
                nc.scalar.activation(sd[:], mv[:, 1:2], AF.Sqrt,
                                     bias=eps_ln_c[:], scale=1.0)
                rstd = p1st.tile([128, 1], F32, name="rstd")
                nc.vector.reciprocal(rstd[:], sd[:])
                nc.vector.tensor_scalar(h0[:], h0[:], mv[:, 0:1], rstd[:],
                                        op0=AOP.subtract, op1=AOP.mult)
                nc.vector.tensor_scalar(h1[:], h1[:], mv[:, 0:1], rstd[:],
                                        op0=AOP.subtract, op1=AOP.mult)
                hts.append((h0, h1))
            for j in range(KT):
                psT = p1psT.tile([128, BT2], BF16, name="psT")
                for tt in range(4):
                    h0, h1 = hts[tt]
                    src = (h0[:, j * 128:(j + 1) * 128] if j < 12 else
                           h1[:, (j - 12) * 128:(j - 11) * 128])
                    nc.tensor.transpose(psT[:, tt * 128:(tt + 1) * 128],
                                        src, ident_bf[:])
                nc.vector.tensor_scalar(nhT_b[:, j, :], psT[:],
                                        scale_cols[:, j:j + 1],
                                        shift_cols[:, j:j + 1],
                                        op0=AOP.mult, op1=AOP.add)
                nc.gpsimd.dma_start(out=nhT_sp[j, :, b * BT2:(b + 1) * BT2],
                                    in_=nhT_b[:, j, :])
            for grp in range(2):          # q features then k features
                psqk = p1ps.tile([128, 3, BT2], F32, name="psqk", tag="pacc")
                for k in range(KT):
                    st, sp = (k == 0), (k == KT - 1)
                    for m in range(3):
                        mm = grp * 3 + m
                        nc.tensor.matmul(psqk[:, m, :],
                                         qkvw_k[k][:, mm * 128:(mm + 1) * 128],
                                         nhT_b[:, k, :], start=st, stop=sp)
                for m in range(3):
                    mm = grp * 3 + m
                    qks = p1ev.tile([128, BT2], BF16, name="qks")
                    nc.vector.tensor_scalar_add(qks[:], psqk[:, m, :],
                                                qkvb_cols[:, mm:mm + 1])
                    nc.sync.dma_start(
                        out=qkT_sp[mm, :, b * BT2:(b + 1) * BT2], in_=qks[:])
            for vh in range(2):           # bank-aligned [2,512] psum groups
                psv = p1ps.tile([128, 2, 512], F32, name="psv", tag="pacc")
                for k in range(KT):
                    st, sp = (k == 0), (k == KT - 1)
                    for mt2 in range(2):
                        mt = vh * 2 + mt2
                        nc.tensor.matmul(psv[:, mt2, 0:QKV],
                                         nhT_b[:, k, mt * 128:(mt + 1) * 128],
                                         qkvw_k[k][:, 768:1152],
                                         start=st, stop=sp)
                for mt2 in range(2):
                    nc.vector.tensor_add(v_sb2[:, b * 4 + vh * 2 + mt2, :],
                                         psv[:, mt2, 0:QKV], vb_b[:])

    # ---------------- Phase 2+3 shared: attnT accumulator ----------------
    with tc.tile_pool(name="attnp", bufs=1) as attnp:
        attnT = attnp.tile([128, HPC, S], BF16)

        # ------------- Phase 2: attention per head (software-pipelined) --
        with tc.tile_pool(name="p2cs", bufs=1) as p2cs, \
             tc.tile_pool(name="p2io", bufs=2) as p2io, \
             tc.tile_pool(name="p2big", bufs=1) as p2big, \
             tc.tile_pool(name="p2sc", bufs=2) as p2sc, \
             tc.tile_pool(name="p2sm", bufs=2) as p2sm, \
             tc.tile_pool(name="p2ex", bufs=2) as p2ex, \
             tc.tile_pool(name="p2ps_s", bufs=3, space="PSUM") as p2ps_s, \
             tc.tile_pool(name="p2ps_a", bufs=2, space="PSUM") as p2ps_a, \
             tc.tile_pool(name="p2ps_m", bufs=3, space="PSUM") as p2ps_m:
            cos_sb = p2cs.tile([128, S], BF16)
            nc.gpsimd.dma_start(out=cos_sb[:], in_=din["cosT"][:, :])
            sin_sb = p2cs.tile([128, S], BF16)
            nc.gpsimd.dma_start(out=sin_sb[:], in_=din["sinT"][:, :])

            def prologue(h):
                qT = p2io.tile([128, S], BF16, name="qT")
                nc.sync.dma_start(out=qT[:], in_=qkT_sp[h, :, :])
                kTt = p2io.tile([128, S], BF16, name="kTt")
                nc.sync.dma_start(out=kTt[:], in_=qkT_sp[HPC + h, :, :])

                rows_r = {}
                for nm, tsrc in (("q", qT), ("k", kTt)):
                    sq = p2big.tile([128, S], F32R, name="sq", tag="sqt")
                    nc.scalar.activation(sq[:], tsrc[:], AF.Square)
                    sd_row = p2sc.tile([1, S], F32, name="sd_row",
                                       tag="sd_row")
                    for n4 in range(4):
                        ms = p2ps_m.tile([1, 512], F32, name="ms",
                                         tag="pmisc")
                        nc.tensor.matmul(ms[:], ones_col[:],
                                         sq[:, n4 * 512:(n4 + 1) * 512],
                                         start=True, stop=True)
                        nc.scalar.activation(
                            sd_row[:, n4 * 512:(n4 + 1) * 512],
                            ms[:], AF.Sqrt, bias=eps_rms_c[:],
                            scale=1.0 / HD)
                    rrow = p2sc.tile([1, S], F32, name="rrow_" + nm,
                                     tag="rrow" + nm)
                    nc.vector.reciprocal(rrow[:], sd_row[:])
                    rows_r[nm] = rrow
                # rstd_k columns via DRAM bounce (hidden by pipelining)
                nc.sync.dma_start(out=rk_b[:], in_=rows_r["k"][:])
                rstdk_cols = p2sc.tile([128, 16], F32, name="rstdk_cols")
                nc.gpsimd.dma_start(
                    out=rstdk_cols[:],
                    in_=rk_b.rearrange("(a p) -> p a", p=128))
                nc.vector.tensor_scalar_mul(rstdk_cols[:], rstdk_cols[:],
                                            1.0 / float(np.sqrt(HD)))

                nc.vector.tensor_scalar_mul(qT[:], qT[:], rmsq_col[:])
                nc.vector.tensor_scalar_mul(kTt[:], kTt[:], rmsk_col[:])

                # q *= rstd_q (rank-1 PE broadcast; commutes with rope)
                for n4 in range(4):
                    n4s = slice(n4 * 512, (n4 + 1) * 512)
                    bq = p2ps_m.tile([128, 512], F32, name="bq", tag="pmisc")
                    nc.tensor.matmul(bq[:], ones_row[:],
                                     rows_r["q"][:, n4s],
                                     start=True, stop=True)
                    nc.vector.tensor_mul(qT[:, n4s], qT[:, n4s], bq[:])

                # rope: out = x*cos + swap(x)*sin_signed
                def rope_sum(dst, srct):
                    sw = p2big.tile([128, S], BF16, name="ropesw",
                                    tag="ropesw")
                    nc.gpsimd.dma_start(out=sw[0:64, :], in_=srct[64:128, :])
                    nc.gpsimd.dma_start(out=sw[64:128, :], in_=srct[0:64, :])
                    t1 = p2big.tile([128, S], BF16, name="ropet1",
                                    tag="ropet1")
                    nc.vector.tensor_mul(t1[:], srct[:], cos_sb[:])
                    nc.vector.tensor_mul(sw[:], sw[:], sin_sb[:])
                    nc.vector.tensor_add(dst[:], t1[:], sw[:])

                qr = p2sc.tile([128, S], BF16, name="qr")
                rope_sum(qr, qT)
                kr = p2sc.tile([128, S], BF16, name="kr")
                rope_sum(kr, kTt)
                return dict(qr=qr, kr=kr, rstdk=rstdk_cols)

            def qc_loop(h, pro):
                qr, kr, rstdk_cols = pro["qr"], pro["kr"], pro["rstdk"]
                for qc in range(4):
                    qsl = slice(qc * 512, (qc + 1) * 512)
                    expS = p2ex.tile([128, 16, 512], BF16, name="expS")
                    for kk in range(16):
                        ps_s = p2ps_s.tile([128, 512], F32, name="ps_s")
                        nc.tensor.matmul(ps_s[:],
                                         kr[:, kk * 128:(kk + 1) * 128],
                                         qr[:, qsl], start=True, stop=True)
                        nc.scalar.activation(expS[:, kk, :], ps_s[:], AF.Exp,
                                             scale=rstdk_cols[:, kk:kk + 1])
                    dtr = p2sm.tile([128, 8, 512], BF16, name="dtr",
                                    tag="dtr")
                    for i in range(8):
                        nc.vector.tensor_add(dtr[:, i, :], expS[:, 2 * i, :],
                                             expS[:, 2 * i + 1, :])
                    for i in range(4):
                        nc.vector.tensor_add(dtr[:, i, :], dtr[:, 2 * i, :],
                                             dtr[:, 2 * i + 1, :])
                    for i in range(2):
                        nc.vector.tensor_add(dtr[:, i, :], dtr[:, 2 * i, :],
                                             dtr[:, 2 * i + 1, :])
                    nc.vector.tensor_add(dtr[:, 0, :], dtr[:, 0, :],
                                         dtr[:, 1, :])
                    ps_d = p2ps_m.tile([1, 512], F32, name="ps_d",
                                       tag="pmisc")
                    nc.tensor.matmul(ps_d[:], ones_col_bf[:], dtr[:, 0, :],
                                     start=True, stop=True)
                    rec_row = p2sm.tile([1, 512], F32, name="rec_row")
                    nc.vector.reciprocal(rec_row[:], ps_d[:])
                    ps_db = p2ps_m.tile([128, 512], F32, name="ps_db",
                                        tag="pmisc")
                    nc.tensor.matmul(ps_db[:], ones_row[:], rec_row[:],
                                     start=True, stop=True)
                    den_sb = p2sm.tile([128, 512], F32, name="den_sb")
                    nc.vector.tensor_copy(den_sb[:], ps_db[:])
                    ps_a = p2ps_a.tile([128, 512], F32, name="ps_a")
                    for kk in range(16):
                        nc.tensor.matmul(
                            ps_a[:], v_sb2[:, kk, h * 128:(h + 1) * 128],
                            expS[:, kk, :],
                            start=(kk == 0), stop=(kk == 15))
                    nc.vector.tensor_mul(attnT[:, h, qsl], ps_a[:],
                                         den_sb[:])

            pros = []
            for h in range(HPC):
                pros.append(prologue(h))
                if h > 0:
                    qc_loop(h - 1, pros[h - 1])
            qc_loop(HPC - 1, pros[HPC - 1])

        # ------------- Phase 3a: MLP hidden (resident weights) -----------
        with tc.tile_pool(name="p3hid", bufs=1) as p3hid:
            hidT = p3hid.tile([128, 12, S], BF16, name="hidT")
            with tc.tile_pool(name="p3nh", bufs=2) as p3nh, \
                 tc.tile_pool(name="p3mw", bufs=1) as p3mw, \
                 tc.tile_pool(name="p3psh", bufs=7,
                              space="PSUM") as p3psh:
                mwk = [p3mw.tile([128, MHC], BF16, name=f"mw{k}")
                       for k in range(KT)]
                for k in range(KT):
                    nc.sync.dma_start(
                        out=mwk[k][:],
                        in_=din["mlpwT"][k * 128:(k + 1) * 128, :])
                for tc4 in range(4):
                    toff = tc4 * 512
                    nhT_c = p3nh.tile([128, KT, 512], BF16, name="nhT_c")
                    nc.gpsimd.dma_start(
                        out=nhT_c[:],
                        in_=nhT_sp[:, :, toff:toff + 512].rearrange(
                            "j p t -> p j t"))
                    for hh in range(2):
                        ps_hs = [p3psh.tile([128, 512], F32, name="ps_h",
                                            tag="psh") for _ in range(6)]
                        for k in range(KT):
                            for m in range(6):
                                nc.tensor.matmul(
                                    ps_hs[m][:],
                                    mwk[k][:, hh * 768 + m * 128:
                                           hh * 768 + (m + 1) * 128],
                                    nhT_c[:, k, :],
                                    start=(k == 0), stop=(k == KT - 1))
                        for m in range(6):
                            idx = hh * 6 + m
                            nc.scalar.activation(
                                hidT[:, idx, toff:toff + 512],
                                ps_hs[m][:], AF.Gelu_apprx_tanh,
                                bias=mlpb_cols[:, idx:idx + 1], scale=1.0)

            # --------- Phase 3b: out-projection + chunked ReduceScatter --
            with tc.tile_pool(name="p3ow", bufs=2) as p3ow, \
                 tc.tile_pool(name="p3ev", bufs=4) as p3ev, \
                 tc.tile_pool(name="p4", bufs=2) as p4, \
                 tc.tile_pool(name="p4c", bufs=1) as p4c, \
                 tc.tile_pool(name="p3pso", bufs=6, space="PSUM") as p3pso:
                gate_b = p4c.tile([128, D], F32)
                g_src = emb_all[2 * D:3 * D]
                nc.gpsimd.dma_start(
                    out=gate_b[:],
                    in_=bass.AP(g_src.tensor, g_src.offset,
                                [[0, 128], [1, D]]))
                outb_b = p4c.tile([128, D], F32)
                ob_src = din["outb"][0:D]
                nc.gpsimd.dma_start(
                    out=outb_b[:],
                    in_=bass.AP(ob_src.tensor, ob_src.offset,
                                [[0, 128], [1, D]]))
                NKO = CAT // 128
                for n6 in range(NC6):
                    ncol = slice(n6 * CW, (n6 + 1) * CW)
                    ow = p3ow.tile([128, NKO, CW], BF16, name="ow")
                    nc.sync.dma_start(
                        out=ow[:],
                        in_=din["outwT"].rearrange(
                            "(ko p) n -> p ko n", p=128)[:, :, ncol])
                    for m16 in range(16):
                        msl = slice(m16 * 128, (m16 + 1) * 128)
                        ps_o = p3pso.tile([128, CW], F32, name="ps_o",
                                          tag="pso")
                        for k in range(NKO):
                            lhsT = (attnT[:, k, msl] if k < HPC else
                                    hidT[:, k - HPC, msl])
                            nc.tensor.matmul(ps_o[:], lhsT, ow[:, k, :],
                                             start=(k == 0),
                                             stop=(k == NKO - 1))
                        po = p3ev.tile([128, CW], BF16, name="po")
                        nc.vector.tensor_copy(po[:], ps_o[:])
                        ri, rc = n6 // GPC, (n6 % GPC) * CW
                        nc.gpsimd.dma_start(
                            out=partial_c[ri][msl, rc:rc + CW], in_=po[:])
                    if n6 % GPC == GPC - 1:
                        ri = n6 // GPC
                        if SIM:
                            nc.sync.dma_start(out=rs_c[ri][:, :],
                                              in_=partial_c[ri][0:SO, :])
                        else:
                            nc.gpsimd.collective_compute(
                                "ReduceScatter", AOP.add,
                                replica_groups=[list(range(NCORES))],
                                ins=[partial_c[ri].opt()],
                                outs=[rs_c[ri].opt()])
                        rsl = slice(ri * RW, (ri + 1) * RW)
                        for t in range(2):
                            tsl = slice(t * 128, (t + 1) * 128)
                            rt = p4.tile([128, RW], BF16, name="rt")
                            nc.sync.dma_start(out=rt[:],
                                              in_=rs_c[ri][tsl, :])
                            ht = p4.tile([128, RW], F32, name="ht")
                            nc.scalar.dma_start(out=ht[:],
                                                in_=hs_res[tsl, rsl])
                            tmp = p4.tile([128, RW], F32, name="tmp")
                            nc.vector.tensor_add(tmp[:], rt[:],
                                                 outb_b[:, rsl])
                            nc.vector.tensor_mul(tmp[:], tmp[:],
                                                 gate_b[:, rsl])
                            nc.vector.tensor_add(tmp[:], tmp[:], ht[:])
                            nc.sync.dma_start(out=out_d[tsl, rsl],
                                              in_=tmp[:])
            if dbg:
                nc.gpsimd.dma_start(out=dbg["dv"][:, :, :], in_=v_sb2[:, :, :])
                nc.gpsimd.dma_start(out=dbg["dattn"][:, :, :],
                                    in_=attnT[:, :, :])
                nc.gpsimd.dma_start(out=dbg["dhid"][:, :, :],
                                    in_=hidT[:, :, 0:256])

    if dbg:
        nc.gpsimd.dma_start(out=dbg["dnh"][:, :, :], in_=nhT_sp[:, :, 0:256])
        nc.gpsimd.dma_start(out=dbg["dqk"][:, :, :], in_=qkT_sp[:, :, 0:256])



_PROG = None


def _get_prog():
    global _PROG
    if _PROG is None:
        _PROG = _build()
    return _PROG


_RUN = None


def _get_runner():
    """Cached jitted SPMD executor (adapted from bass2jax.run_bass_via_pjrt)
    so repeated calls reuse the compiled NEFF for steady-state timing."""
    global _RUN
    if _RUN is not None:
        return _RUN
    import jax
    from jax.experimental.shard_map import shard_map
    from jax.sharding import Mesh, PartitionSpec
    from concourse import bass2jax

    nc = _get_prog()
    bass2jax.install_neuronx_cc_hook()
    partition_name = (nc.partition_id_tensor.name
                      if nc.partition_id_tensor else None)
    in_names, out_names, out_avals, zero_outs = [], [], [], []
    in_avals = []
    for alloc in nc.m.functions[0].allocations:
        if not isinstance(alloc, mybir.MemoryLocationSet):
            continue
        name = alloc.memorylocations[0].name
        if alloc.kind == "ExternalInput":
            if name != partition_name:
                in_names.append(name)
                in_avals.append(jax.core.ShapedArray(
                    tuple(alloc.tensor_shape), mybir.dt.np(alloc.dtype)))
        elif alloc.kind == "ExternalOutput":
            shape = tuple(alloc.tensor_shape)
            dtype = mybir.dt.np(alloc.dtype)
            out_names.append(name)
            out_avals.append(jax.core.ShapedArray(shape, dtype))
            zero_outs.append(np.zeros(shape, dtype))
    n_params = len(in_names)
    n_outs = len(out_avals)
    in_names = in_names + out_names
    if partition_name is not None:
        in_names.append(partition_name)
    donate = tuple(range(n_params, n_params + n_outs))

    def _body(*args):
        operands = list(args)
        if partition_name is not None:
            operands.append(bass2jax.partition_id_tensor())
        outs = bass2jax._bass_exec_p.bind(
            *operands,
            out_avals=tuple(out_avals),
            in_names=tuple(in_names),
            out_names=tuple(out_names),
            lowering_input_output_aliases=(),
            sim_require_finite=True,
            sim_require_nnan=True,
            nc=nc,
        )
        return tuple(outs)

    devices = jax.devices()[:NCORES]
    mesh = Mesh(np.asarray(devices), ("core",))
    in_specs = (PartitionSpec("core"),) * (n_params + n_outs)
    out_specs = (PartitionSpec("core"),) * n_outs

    global_avals = [
        jax.ShapeDtypeStruct((NCORES * a.shape[0], *a.shape[1:]), a.dtype)
        for a in in_avals + out_avals]

    def _compile_fn():
        jitted = jax.jit(
            shard_map(_body, mesh=mesh, in_specs=in_specs,
                      out_specs=out_specs, check_rep=False),
            donate_argnums=donate, keep_unused=True)
        return jitted.lower(*global_avals).compile()

    try:
        sharded = bass2jax.fast_dispatch_compile(_compile_fn)
    except Exception:
        sharded = jax.jit(
            shard_map(_body, mesh=mesh, in_specs=in_specs,
                      out_specs=out_specs, check_rep=False),
            donate_argnums=donate, keep_unused=True)
    _RUN = dict(fn=sharded, in_names=in_names, out_names=out_names,
                out_avals=out_avals, zero_outs=zero_outs, n_params=n_params,
                mesh=mesh)
    return _RUN


PIPE_N = 100


def _run_spmd(maps, time_iters=0):
    import jax
    from jax.sharding import NamedSharding, PartitionSpec
    import time as _time
    r = _get_runner()
    names = r["in_names"][:r["n_params"]]
    concat_in = [np.concatenate([np.asarray(maps[c][nm]) for c in
                                 range(NCORES)], axis=0) for nm in names]
    sh = NamedSharding(r["mesh"], PartitionSpec("core"))
    dev_in = [jax.device_put(a, sh) for a in concat_in]
    for a in dev_in:
        a.block_until_ready()

    zeros = [np.zeros((NCORES * z.shape[0], *z.shape[1:]), z.dtype)
             for z in r["zero_outs"]]
    # The kernel fully overwrites every ExternalOutput element, so each
    # timed call donates the previous call's output buffers: the chain
    # serializes executions on-device while the host streams dispatches.
    outs = r["fn"](*dev_in, *zeros)
    jax.block_until_ready(outs)
    times = []
    if time_iters:
        for _ in range(5):
            outs = r["fn"](*dev_in, *outs)
        jax.block_until_ready(outs)
        for _ in range(time_iters):
            t0 = _time.perf_counter()
            for _ in range(PIPE_N):
                outs = r["fn"](*dev_in, *outs)
            jax.block_until_ready(outs)
            times.append((_time.perf_counter() - t0) / PIPE_N)
    host = [np.asarray(a) for a in outs]
    res = [{nm: host[i].reshape(NCORES, *r["out_avals"][i].shape)[c]
            for i, nm in enumerate(r["out_names"])}
           for c in range(NCORES)]
    return res, times


def _shards(inputs):
    f = lambda x: np.ascontiguousarray(np.asarray(x), dtype=np.float32)
    hs2 = f(inputs["hidden_states"]).reshape(S, D)
    temb = f(inputs["temb"]).reshape(D)
    pi = np.concatenate([np.arange(0, HD, 2), np.arange(1, HD, 2)])
    cosp = f(np.asarray(inputs["rope_cos"])[:, pi].T)
    sinp = f(np.asarray(inputs["rope_sin"])[:, pi].T)
    sinp[0:64, :] *= -1.0
    q_w = f(inputs["q_w"]).reshape(HEADS, HD, D)[:, pi, :]
    k_w = f(inputs["k_w"]).reshape(HEADS, HD, D)[:, pi, :]
    v_w = f(inputs["v_w"])
    q_b = f(inputs["q_b"]).reshape(HEADS, HD)[:, pi]
    k_b = f(inputs["k_b"]).reshape(HEADS, HD)[:, pi]
    v_b = f(inputs["v_b"])
    mlp_w, mlp_b = f(inputs["mlp_w"]), f(inputs["mlp_b"])
    out_w, out_b = f(inputs["out_w"]), f(inputs["out_b"])
    norm_w, norm_b = f(inputs["norm_w"]), f(inputs["norm_b"])
    rmsq, rmsk = f(inputs["rms_q_w"])[pi], f(inputs["rms_k_w"])[pi]
    ident = np.eye(128, dtype=np.float32)

    maps = []
    for c in range(NCORES):
        hsl = slice(c * HPC, (c + 1) * HPC)
        vsl = slice(c * QKV, (c + 1) * QKV)
        msl = slice(c * MHC, (c + 1) * MHC)
        esl = slice(c * EMBC, (c + 1) * EMBC)
        qkvwT = np.ascontiguousarray(np.concatenate([
            q_w[hsl].reshape(QKV, D).T,
            k_w[hsl].reshape(QKV, D).T,
            v_w[vsl].T], axis=1)).astype(ml_dtypes.bfloat16)
        qkvb = np.concatenate([q_b[hsl].ravel(), k_b[hsl].ravel(), v_b[vsl]])
        outwT = np.ascontiguousarray(np.concatenate([
            out_w[:, vsl].T,
            out_w[:, D + c * MHC:D + (c + 1) * MHC].T], axis=0)).astype(
                ml_dtypes.bfloat16)
        maps.append({
            "hs": hs2,
            "hs_res": np.ascontiguousarray(hs2[c * SO:(c + 1) * SO]),
            "temb": temb, "cosT": cosp, "sinT": sinp,
            "qkvwT": qkvwT, "qkvb": np.ascontiguousarray(qkvb),
            "mlpwT": np.ascontiguousarray(mlp_w[msl].T).astype(
                ml_dtypes.bfloat16),
            "mlpb": np.ascontiguousarray(mlp_b[msl]),
            "outwT": outwT, "outb": out_b,
            "nwT": np.ascontiguousarray(norm_w[esl].T).astype(
                ml_dtypes.bfloat16),
            "nb": np.ascontiguousarray(norm_b[esl]),
            "rmsq": np.ascontiguousarray(rmsq),
            "rmsk": np.ascontiguousarray(rmsk),
            "ident": ident,
        })
    return maps


def kernel(**inputs):
    maps = _shards(inputs)
    res, times = _run_spmd(maps, time_iters=TIME_ITERS)
    LAST["results"] = res
    LAST["times"] = times
    out = np.concatenate([res[c]["out"] for c in range(NCORES)], axis=0)
    return out.reshape(1, S, D)



# revision 33
# speedup vs baseline: 1.3989x; 1.3989x over previous
"""Trainium2 Bass kernel for BriaFibo single transformer block.

Tensor-parallel over 8 NeuronCores: heads (24 -> 3/core) and mlp_hidden
(12288 -> 1536/core) are column-sharded; out projection row-sharded with a
device-side ReduceScatter.  AdaLN emb matvec is row-sharded + AllGather.
All big matmuls run in float32r (full PE rate at N>=256, ~fp32 storage).
"""

import ml_dtypes
import numpy as np

import concourse.bass as bass
import concourse.mybir as mybir
import concourse.tile as tile
from concourse import bacc
from concourse.bass_utils import run_bass_kernel_spmd

F32 = mybir.dt.float32
F32R = mybir.dt.float32r
BF16 = mybir.dt.bfloat16
AOP = mybir.AluOpType
AF = mybir.ActivationFunctionType

S, D = 2048, 3072
HEADS, HD = 24, 128
MH = 12288
NCORES = 8
HPC = HEADS // NCORES          # 3 heads/core
QKV = HPC * HD                 # 384
MHC = MH // NCORES             # 1536
CAT = QKV + MHC                # 1920
SO = S // NCORES               # 256 output rows/core
KT = D // 128                  # 24 contraction tiles
EMBC = 3 * D // NCORES         # 1152 adaLN rows/core
EPS_LN = 1e-6
EPS_RMS = 1e-6

TRACE = False
RSCHUNKS = 6
TIME_ITERS = 0
DEBUG = False
SIM = False
LAST = {}


def _r(ap):
    return ap.bitcast(F32R)



def _build():
    nc = bacc.Bacc("TRN2", target_bir_lowering=False, debug=False,
                   num_devices=NCORES)

    din = {}
    for name, shape, dt in [
        ("hs", [S, D], F32), ("hs_res", [SO, D], F32), ("temb", [D], F32),
        ("cosT", [HD, S], F32), ("sinT", [HD, S], F32),
        ("qkvwT", [D, 3 * QKV], BF16), ("qkvb", [3 * QKV], F32),
        ("mlpwT", [D, MHC], BF16), ("mlpb", [MHC], F32),
        ("outwT", [CAT, D], BF16), ("outb", [D], F32),
        ("nwT", [D, EMBC], BF16), ("nb", [EMBC], F32),
        ("rmsq", [HD], F32), ("rmsk", [HD], F32), ("ident", [128, 128], F32),
    ]:
        din[name] = nc.dram_tensor(name, shape, dt, kind="ExternalInput")
    out_d = nc.dram_tensor("out", [SO, D], F32, kind="ExternalOutput")
    dbg = {}
    if DEBUG:
        for name, shape in [("demb", [3 * D]), ("dnh", [KT, 128, 256]),
                            ("dqk", [2 * HPC, 128, 256]),
                            ("dv", [128, S // 128, QKV]),
                            ("dattn", [128, HPC, S]),
                            ("dhid", [128, 12, 256])]:
            dbg[name] = nc.dram_tensor(name, shape, F32,
                                       kind="ExternalOutput")

    from contextlib import ExitStack
    with tile.TileContext(nc) as tc, ExitStack() as ctx:
        _emit(ctx, nc, tc, din, out_d, dbg)
    nc.compile()
    return nc


def _emit(ctx, nc, tc, din, out_d, dbg=None):
    hs, hs_res = din["hs"], din["hs_res"]

    cpool = ctx.enter_context(tc.tile_pool(name="consts", bufs=1))
    dram = ctx.enter_context(tc.tile_pool(name="dram", bufs=1, space="DRAM"))

    ident_sb = cpool.tile([128, 128], F32)
    nc.sync.dma_start(out=ident_sb[:], in_=din["ident"][:, :])
    ident_bf = cpool.tile([128, 128], BF16)
    nc.gpsimd.dma_start(out=ident_bf[:], in_=din["ident"][:, :])
    ones_f = cpool.tile([128, 128], F32)
    nc.vector.memset(ones_f[:], 1.0)
    ones_col = cpool.tile([128, 1], F32R)         # lhsT for colsum -> [1,N]
    nc.vector.tensor_copy(ones_col[:], ones_f[:, 0:1])
    ones_col_bf = cpool.tile([128, 1], BF16)      # bf16 colsum lhsT
    nc.vector.tensor_copy(ones_col_bf[:], ones_f[:, 0:1])
    ones_row = cpool.tile([1, 128], F32)          # lhsT for bcast -> [128,N]
    nc.vector.tensor_copy(ones_row[:], ones_f[0:1, :])
    eps_ln_c = cpool.tile([128, 1], F32)
    nc.vector.memset(eps_ln_c[:], EPS_LN)
    eps_rms_c = cpool.tile([1, 1], F32)
    nc.vector.memset(eps_rms_c[:], EPS_RMS)

    rmsq_col = cpool.tile([128, 1], F32)
    nc.gpsimd.dma_start(out=rmsq_col[:],
                        in_=din["rmsq"].rearrange("(p one) -> p one", one=1))
    rmsk_col = cpool.tile([128, 1], F32)
    nc.gpsimd.dma_start(out=rmsk_col[:],
                        in_=din["rmsk"].rearrange("(p one) -> p one", one=1))
    qkvb_cols = cpool.tile([128, 9], F32)
    nc.gpsimd.dma_start(out=qkvb_cols[:],
                        in_=din["qkvb"].rearrange("(m p) -> p m", p=128))
    vb_b = cpool.tile([128, QKV], F32)
    vb_src = din["qkvb"][768:1152]
    nc.gpsimd.dma_start(
        out=vb_b[:],
        in_=bass.AP(vb_src.tensor, vb_src.offset, [[0, 128], [1, QKV]]))
    mlpb_cols = cpool.tile([128, 12], F32)
    nc.gpsimd.dma_start(out=mlpb_cols[:],
                        in_=din["mlpb"].rearrange("(m p) -> p m", p=128))

    # DRAM scratch
    nhT_sp = dram.tile([KT, 128, S], BF16)
    qkT_sp = dram.tile([2 * HPC, 128, S], BF16)
    ag_in = dram.tile([EMBC], F32)
    rk_b = dram.tile([S], F32)
    emb_all = dram.tile([3 * D], F32, addr_space="Shared")
    NC6 = 6                                     # out-proj column chunks
    CW = D // NC6                               # 512 cols per chunk
    RSCH = globals().get("RSCHUNKS", RSCHUNKS)  # collectives count
    RW = D // RSCH
    GPC = NC6 // RSCH                           # compute chunks per RS
    partial_c = [dram.tile([S, RW], BF16, name=f"partial{i}")
                 for i in range(RSCH)]
    rs_c = [dram.tile([SO, RW], BF16, name=f"rsc{i}") for i in range(RSCH)]

    # ---------------- Phase 0: AdaLN emb (sharded matvec + AllGather) ----
    with tc.tile_pool(name="p0", bufs=1) as p0, \
         tc.tile_pool(name="p0st", bufs=3) as p0st, \
         tc.tile_pool(name="p0ps", bufs=1, space="PSUM") as p0ps:
        temb_sb = p0.tile([128, KT], F32)
        nc.gpsimd.dma_start(out=temb_sb[:],
                            in_=din["temb"].rearrange("(a p) -> p a", p=128))
        silu_t = p0.tile([128, KT], BF16)
        nc.scalar.activation(silu_t[:], temb_sb[:], AF.Silu)
        pe_all = p0ps.tile([1, 3, 512], F32)
        for k in range(KT):
            nw_k = p0st.tile([128, EMBC], BF16, name="nw_k")
            nc.sync.dma_start(out=nw_k[:],
                              in_=din["nwT"][k * 128:(k + 1) * 128, :])
            for n in range(3):
                nc.tensor.matmul(pe_all[:, n, 0:384],
                                 silu_t[:, k:k + 1],
                                 nw_k[:, n * 384:(n + 1) * 384],
                                 start=(k == 0), stop=(k == KT - 1))
        nb_sb = p0.tile([1, EMBC], F32)
        nc.sync.dma_start(out=nb_sb[:],
                          in_=din["nb"].rearrange("(one a) -> one a", one=1))
        emb_row = p0.tile([1, EMBC], F32)
        for n in range(3):
            nc.vector.tensor_add(emb_row[:, n * 384:(n + 1) * 384],
                                 pe_all[:, n, 0:384],
                                 nb_sb[:, n * 384:(n + 1) * 384])
        nc.sync.dma_start(out=ag_in[:], in_=emb_row[:])
        if SIM:
            nc.sync.dma_start(out=emb_all[0:EMBC], in_=ag_in[:])
        else:
            nc.gpsimd.collective_compute(
                "AllGather", AOP.bypass,
                replica_groups=[list(range(NCORES))],
                ins=[ag_in.opt()], outs=[emb_all.opt()])

    if dbg:
        nc.sync.dma_start(out=dbg["demb"][:], in_=emb_all[:])

    scale_cols = cpool.tile([128, KT], F32)
    sc_src = emb_all[D:2 * D]
    nc.gpsimd.dma_start(
        out=scale_cols[:],
        in_=bass.AP(sc_src.tensor, sc_src.offset, [[1, 128], [128, KT]]))
    nc.vector.tensor_scalar_add(scale_cols[:], scale_cols[:], 1.0)
    shift_cols = cpool.tile([128, KT], F32)
    sh_src = emb_all[0:D]
    nc.gpsimd.dma_start(
        out=shift_cols[:],
        in_=bass.AP(sh_src.tensor, sh_src.offset, [[1, 128], [128, KT]]))

    # V stays SBUF-resident (bf16) from phase 1 through attention
    vres = ctx.enter_context(tc.tile_pool(name="vres", bufs=1))
    v_sb2 = vres.tile([128, S // 128, QKV], BF16)

    # ---------------- Phase 1: LN + transpose + qkv/v projections --------
    # bf16 pipeline, 512-token blocks, resident bf16 qkv weights
    NB2, BT2 = 4, 512
    with tc.tile_pool(name="p1w", bufs=1) as p1w, \
         tc.tile_pool(name="p1hs", bufs=2) as p1hs, \
         tc.tile_pool(name="p1st", bufs=3) as p1st, \
         tc.tile_pool(name="p1nh", bufs=2) as p1nh, \
         tc.tile_pool(name="p1ev", bufs=3) as p1ev, \
         tc.tile_pool(name="p1ps", bufs=2, space="PSUM") as p1ps, \
         tc.tile_pool(name="p1psT", bufs=2, space="PSUM") as p1psT:
        qkvw_k = [p1w.tile([128, 3 * QKV], BF16, name=f"qw{k}")
                  for k in range(KT)]
        for k in range(KT):
            nc.sync.dma_start(out=qkvw_k[k][:],
                              in_=din["qkvwT"][k * 128:(k + 1) * 128, :])
        for b in range(NB2):
            nhT_b = p1nh.tile([128, KT, BT2], BF16, name="nhT_b")
            hts = []
            for tt in range(4):
                row = b * BT2 + tt * 128
                h0 = p1hs.tile([128, D // 2], BF16, name=f"h{tt}a",
                               tag=f"h{tt}a")
                nc.gpsimd.dma_start(out=h0[:],
                                    in_=hs[row:row + 128, 0:D // 2])
                h1 = p1hs.tile([128, D // 2], BF16, name=f"h{tt}b",
                               tag=f"h{tt}b")
                nc.gpsimd.dma_start(out=h1[:],
                                    in_=hs[row:row + 128, D // 2:D])
                stats = p1st.tile([128, 6, 6], F32, name="stats")
                for g in range(3):
                    nc.vector.bn_stats(stats[:, g, :],
                                       h0[:, g * 512:(g + 1) * 512])
                    nc.vector.bn_stats(stats[:, 3 + g, :],
                                       h1[:, g * 512:(g + 1) * 512])
                mv = p1st.tile([128, 2], F32, name="mv")
                nc.vector.bn_aggr(mv[:], stats[:])
                sd = p1st.tile([128, 1], F32, name="sd")
                nc.scalar.activation(sd[:], mv[:, 1:2], AF.Sqrt,
                                     bias=eps_ln_c[:], scale=1.0)
                rstd = p1st.tile([128, 1], F32, name="rstd")
                nc.vector.reciprocal(rstd[:], sd[:])
                nc.vector.tensor_scalar(h0[:], h0[:], mv[:, 0:1], rstd[:],
                                        op0=AOP.subtract, op1=AOP.mult)
                nc.vector.tensor_scalar(h1[:], h1[:], mv[:, 0:1], rstd[:],
                                        op0=AOP.subtract, op1=AOP.mult)
                hts.append((h0, h1))
            for j in range(KT):
                psT = p1psT.tile([128, BT2], BF16, name="psT")
                for tt in range(4):
                    h0, h1 = hts[tt]
                    src = (h0[:, j * 128:(j + 1) * 128] if j < 12 else
                           h1[:, (j - 12) * 128:(j - 11) * 128])
                    nc.tensor.transpose(psT[:, tt * 128:(tt + 1) * 128],
                                        src, ident_bf[:])
                nc.vector.tensor_scalar(nhT_b[:, j, :], psT[:],
                                        scale_cols[:, j:j + 1],
                                        shift_cols[:, j:j + 1],
                                        op0=AOP.mult, op1=AOP.add)
                nc.gpsimd.dma_start(out=nhT_sp[j, :, b * BT2:(b + 1) * BT2],
                                    in_=nhT_b[:, j, :])
            for grp in range(2):          # q features then k features
                psqk = p1ps.tile([128, 3, BT2], F32, name="psqk", tag="pacc")
                for k in range(KT):
                    st, sp = (k == 0), (k == KT - 1)
                    for m in range(3):
                        mm = grp * 3 + m
                        nc.tensor.matmul(psqk[:, m, :],
                                         qkvw_k[k][:, mm * 128:(mm + 1) * 128],
                                         nhT_b[:, k, :], start=st, stop=sp)
                for m in range(3):
                    mm = grp * 3 + m
                    qks = p1ev.tile([128, BT2], BF16, name="qks")
                    nc.vector.tensor_scalar_add(qks[:], psqk[:, m, :],
                                                qkvb_cols[:, mm:mm + 1])
                    nc.sync.dma_start(
                        out=qkT_sp[mm, :, b * BT2:(b + 1) * BT2], in_=qks[:])
            for vh in range(2):           # bank-aligned [2,512] psum groups
                psv = p1ps.tile([128, 2, 512], F32, name="psv", tag="pacc")
                for k in range(KT):
                    st, sp = (k == 0), (k == KT - 1)
                    for mt2 in range(2):
                        mt = vh * 2 + mt2
                        nc.tensor.matmul(psv[:, mt2, 0:QKV],
                                         nhT_b[:, k, mt * 128:(mt + 1) * 128],
                                         qkvw_k[k][:, 768:1152],
                                         start=st, stop=sp)
                for mt2 in range(2):
                    nc.vector.tensor_add(v_sb2[:, b * 4 + vh * 2 + mt2, :],
                                         psv[:, mt2, 0:QKV], vb_b[:])

    # ---------------- Phase 2+3 shared: attnT accumulator ----------------
    with tc.tile_pool(name="attnp", bufs=1) as attnp:
        attnT = attnp.tile([128, HPC, S], BF16)

        # ------------- Phase 2: attention per head (software-pipelined) --
        with tc.tile_pool(name="p2cs", bufs=1) as p2cs, \
             tc.tile_pool(name="p2io", bufs=2) as p2io, \
             tc.tile_pool(name="p2big", bufs=1) as p2big, \
             tc.tile_pool(name="p2sc", bufs=2) as p2sc, \
             tc.tile_pool(name="p2sm", bufs=2) as p2sm, \
             tc.tile_pool(name="p2ex", bufs=2) as p2ex, \
             tc.tile_pool(name="p2ps_s", bufs=3, space="PSUM") as p2ps_s, \
             tc.tile_pool(name="p2ps_a", bufs=2, space="PSUM") as p2ps_a, \
             tc.tile_pool(name="p2ps_m", bufs=3, space="PSUM") as p2ps_m:
            cos_sb = p2cs.tile([128, S], BF16)
            nc.gpsimd.dma_start(out=cos_sb[:], in_=din["cosT"][:, :])
            sin_sb = p2cs.tile([128, S], BF16)
            nc.gpsimd.dma_start(out=sin_sb[:], in_=din["sinT"][:, :])

            def prologue(h):
                qT = p2io.tile([128, S], BF16, name="qT")
                nc.sync.dma_start(out=qT[:], in_=qkT_sp[h, :, :])
                kTt = p2io.tile([128, S], BF16, name="kTt")
                nc.sync.dma_start(out=kTt[:], in_=qkT_sp[HPC + h, :, :])

                rows_r = {}
                for nm, tsrc in (("q", qT), ("k", kTt)):
                    sq = p2big.tile([128, S], F32R, name="sq", tag="sqt")
                    nc.scalar.activation(sq[:], tsrc[:], AF.Square)
                    sd_row = p2sc.tile([1, S], F32, name="sd_row",
                                       tag="sd_row")
                    for n4 in range(4):
                        ms = p2ps_m.tile([1, 512], F32, name="ms",
                                         tag="pmisc")
                        nc.tensor.matmul(ms[:], ones_col[:],
                                         sq[:, n4 * 512:(n4 + 1) * 512],
                                         start=True, stop=True)
                        nc.scalar.activation(
                            sd_row[:, n4 * 512:(n4 + 1) * 512],
                            ms[:], AF.Sqrt, bias=eps_rms_c[:],
                            scale=1.0 / HD)
                    rrow = p2sc.tile([1, S], F32, name="rrow_" + nm,
                                     tag="rrow" + nm)
                    nc.vector.reciprocal(rrow[:], sd_row[:])
                    rows_r[nm] = rrow
                # rstd_k columns via DRAM bounce (hidden by pipelining)
                nc.sync.dma_start(out=rk_b[:], in_=rows_r["k"][:])
                rstdk_cols = p2sc.tile([128, 16], F32, name="rstdk_cols")
                nc.gpsimd.dma_start(
                    out=rstdk_cols[:],
                    in_=rk_b.rearrange("(a p) -> p a", p=128))
                nc.vector.tensor_scalar_mul(rstdk_cols[:], rstdk_cols[:],
                                            1.0 / float(np.sqrt(HD)))

                nc.vector.tensor_scalar_mul(qT[:], qT[:], rmsq_col[:])
                nc.vector.tensor_scalar_mul(kTt[:], kTt[:], rmsk_col[:])

                # q *= rstd_q (rank-1 PE broadcast; commutes with rope)
                for n4 in range(4):
                    n4s = slice(n4 * 512, (n4 + 1) * 512)
                    bq = p2ps_m.tile([128, 512], F32, name="bq", tag="pmisc")
                    nc.tensor.matmul(bq[:], ones_row[:],
                                     rows_r["q"][:, n4s],
                                     start=True, stop=True)
                    nc.vector.tensor_mul(qT[:, n4s], qT[:, n4s], bq[:])

                # rope: out = x*cos + swap(x)*sin_signed
                def rope_sum(dst, srct):
                    sw = p2big.tile([128, S], BF16, name="ropesw",
                                    tag="ropesw")
                    nc.gpsimd.dma_start(out=sw[0:64, :], in_=srct[64:128, :])
                    nc.gpsimd.dma_start(out=sw[64:128, :], in_=srct[0:64, :])
                    t1 = p2big.tile([128, S], BF16, name="ropet1",
                                    tag="ropet1")
                    nc.vector.tensor_mul(t1[:], srct[:], cos_sb[:])
                    nc.vector.tensor_mul(sw[:], sw[:], sin_sb[:])
                    nc.vector.tensor_add(dst[:], t1[:], sw[:])

                qr = p2sc.tile([128, S], BF16, name="qr")
                rope_sum(qr, qT)
                kr = p2sc.tile([128, S], BF16, name="kr")
                rope_sum(kr, kTt)
                return dict(qr=qr, kr=kr, rstdk=rstdk_cols)

            def qc_loop(h, pro):
                qr, kr, rstdk_cols = pro["qr"], pro["kr"], pro["rstdk"]
                for qc in range(4):
                    qsl = slice(qc * 512, (qc + 1) * 512)
                    expS = p2ex.tile([128, 16, 512], BF16, name="expS")
                    for kk in range(16):
                        ps_s = p2ps_s.tile([128, 512], F32, name="ps_s")
                        nc.tensor.matmul(ps_s[:],
                                         kr[:, kk * 128:(kk + 1) * 128],
                                         qr[:, qsl], start=True, stop=True)
                        nc.scalar.activation(expS[:, kk, :], ps_s[:], AF.Exp,
                                             scale=rstdk_cols[:, kk:kk + 1])
                    dtr = p2sm.tile([128, 8, 512], BF16, name="dtr",
                                    tag="dtr")
                    for i in range(8):
                        nc.vector.tensor_add(dtr[:, i, :], expS[:, 2 * i, :],
                                             expS[:, 2 * i + 1, :])
                    for i in range(4):
                        nc.vector.tensor_add(dtr[:, i, :], dtr[:, 2 * i, :],
                                             dtr[:, 2 * i + 1, :])
                    for i in range(2):
                        nc.vector.tensor_add(dtr[:, i, :], dtr[:, 2 * i, :],
                                             dtr[:, 2 * i + 1, :])
                    nc.vector.tensor_add(dtr[:, 0, :], dtr[:, 0, :],
                                         dtr[:, 1, :])
                    ps_d = p2ps_m.tile([1, 512], F32, name="ps_d",
                                       tag="pmisc")
                    nc.tensor.matmul(ps_d[:], ones_col_bf[:], dtr[:, 0, :],
                                     start=True, stop=True)
                    rec_row = p2sm.tile([1, 512], F32, name="rec_row")
                    nc.vector.reciprocal(rec_row[:], ps_d[:])
                    ps_db = p2ps_m.tile([128, 512], F32, name="ps_db",
                                        tag="pmisc")
                    nc.tensor.matmul(ps_db[:], ones_row[:], rec_row[:],
                                     start=True, stop=True)
                    den_sb = p2sm.tile([128, 512], F32, name="den_sb")
                    nc.vector.tensor_copy(den_sb[:], ps_db[:])
                    ps_a = p2ps_a.tile([128, 512], F32, name="ps_a")
                    for kk in range(16):
                        nc.tensor.matmul(
                            ps_a[:], v_sb2[:, kk, h * 128:(h + 1) * 128],
                            expS[:, kk, :],
                            start=(kk == 0), stop=(kk == 15))
                    nc.vector.tensor_mul(attnT[:, h, qsl], ps_a[:],
                                         den_sb[:])

            pros = []
            for h in range(HPC):
                pros.append(prologue(h))
                if h > 0:
                    qc_loop(h - 1, pros[h - 1])
            qc_loop(HPC - 1, pros[HPC - 1])

        # ------------- Phase 3a: MLP hidden (resident weights) -----------
        with tc.tile_pool(name="p3hid", bufs=1) as p3hid:
            hidT = p3hid.tile([128, 12, S], BF16, name="hidT")
            with tc.tile_pool(name="p3nh", bufs=2) as p3nh, \
                 tc.tile_pool(name="p3mw", bufs=1) as p3mw, \
                 tc.tile_pool(name="p3psh", bufs=7,
                              space="PSUM") as p3psh:
                mwk = [p3mw.tile([128, MHC], BF16, name=f"mw{k}")
                       for k in range(KT)]
                for k in range(KT):
                    nc.sync.dma_start(
                        out=mwk[k][:],
                        in_=din["mlpwT"][k * 128:(k + 1) * 128, :])
                for tc4 in range(4):
                    toff = tc4 * 512
                    nhT_c = p3nh.tile([128, KT, 512], BF16, name="nhT_c")
                    nc.gpsimd.dma_start(
                        out=nhT_c[:],
                        in_=nhT_sp[:, :, toff:toff + 512].rearrange(
                            "j p t -> p j t"))
                    for hh in range(2):
                        ps_hs = [p3psh.tile([128, 512], F32, name="ps_h",
                                            tag="psh") for _ in range(6)]
                        for k in range(KT):
                            for m in range(6):
                                nc.tensor.matmul(
                                    ps_hs[m][:],
                                    mwk[k][:, hh * 768 + m * 128:
                                           hh * 768 + (m + 1) * 128],
                                    nhT_c[:, k, :],
                                    start=(k == 0), stop=(k == KT - 1))
                        for m in range(6):
                            idx = hh * 6 + m
                            nc.scalar.activation(
                                hidT[:, idx, toff:toff + 512],
                                ps_hs[m][:], AF.Gelu_apprx_tanh,
                                bias=mlpb_cols[:, idx:idx + 1], scale=1.0)

            # --------- Phase 3b: out-projection + chunked ReduceScatter --
            with tc.tile_pool(name="p3ow", bufs=2) as p3ow, \
                 tc.tile_pool(name="p3ev", bufs=4) as p3ev, \
                 tc.tile_pool(name="p4", bufs=2) as p4, \
                 tc.tile_pool(name="p4c", bufs=1) as p4c, \
                 tc.tile_pool(name="p3pso", bufs=6, space="PSUM") as p3pso:
                gate_b = p4c.tile([128, D], F32)
                g_src = emb_all[2 * D:3 * D]
                nc.gpsimd.dma_start(
                    out=gate_b[:],
                    in_=bass.AP(g_src.tensor, g_src.offset,
                                [[0, 128], [1, D]]))
                outb_b = p4c.tile([128, D], F32)
                ob_src = din["outb"][0:D]
                nc.gpsimd.dma_start(
                    out=outb_b[:],
                    in_=bass.AP(ob_src.tensor, ob_src.offset,
                                [[0, 128], [1, D]]))
                NKO = CAT // 128
                for n6 in range(NC6):
                    ncol = slice(n6 * CW, (n6 + 1) * CW)
                    ow = p3ow.tile([128, NKO, CW], BF16, name="ow")
                    nc.sync.dma_start(
                        out=ow[:],
                        in_=din["outwT"].rearrange(
                            "(ko p) n -> p ko n", p=128)[:, :, ncol])
                    for m16 in range(16):
                        msl = slice(m16 * 128, (m16 + 1) * 128)
                        ps_o = p3pso.tile([128, CW], F32, name="ps_o",
                                          tag="pso")
                        for k in range(NKO):
                            lhsT = (attnT[:, k, msl] if k < HPC else
                                    hidT[:, k - HPC, msl])
                            nc.tensor.matmul(ps_o[:], lhsT, ow[:, k, :],
                                             start=(k == 0),
                                             stop=(k == NKO - 1))
                        po = p3ev.tile([128, CW], BF16, name="po")
                        nc.vector.tensor_copy(po[:], ps_o[:])
                        ri, rc = n6 // GPC, (n6 % GPC) * CW
                        nc.gpsimd.dma_start(
                            out=partial_c[ri][msl, rc:rc + CW], in_=po[:])
                    if n6 % GPC == GPC - 1:
                        ri = n6 // GPC
                        if SIM:
                            nc.sync.dma_start(out=rs_c[ri][:, :],
                                              in_=partial_c[ri][0:SO, :])
                        else:
                            nc.gpsimd.collective_compute(
                                "ReduceScatter", AOP.add,
                                replica_groups=[list(range(NCORES))],
                                ins=[partial_c[ri].opt()],
                                outs=[rs_c[ri].opt()])
                        rsl = slice(ri * RW, (ri + 1) * RW)
                        for t in range(2):
                            tsl = slice(t * 128, (t + 1) * 128)
                            rt = p4.tile([128, RW], BF16, name="rt")
                            nc.sync.dma_start(out=rt[:],
                                              in_=rs_c[ri][tsl, :])
                            ht = p4.tile([128, RW], F32, name="ht")
                            nc.scalar.dma_start(out=ht[:],
                                                in_=hs_res[tsl, rsl])
                            tmp = p4.tile([128, RW], F32, name="tmp")
                            nc.vector.tensor_add(tmp[:], rt[:],
                                                 outb_b[:, rsl])
                            nc.vector.tensor_mul(tmp[:], tmp[:],
                                                 gate_b[:, rsl])
                            nc.vector.tensor_add(tmp[:], tmp[:], ht[:])
                            nc.sync.dma_start(out=out_d[tsl, rsl],
                                              in_=tmp[:])
            if dbg:
                nc.gpsimd.dma_start(out=dbg["dv"][:, :, :], in_=v_sb2[:, :, :])
                nc.gpsimd.dma_start(out=dbg["dattn"][:, :, :],
                                    in_=attnT[:, :, :])
                nc.gpsimd.dma_start(out=dbg["dhid"][:, :, :],
                                    in_=hidT[:, :, 0:256])

    if dbg:
        nc.gpsimd.dma_start(out=dbg["dnh"][:, :, :], in_=nhT_sp[:, :, 0:256])
        nc.gpsimd.dma_start(out=dbg["dqk"][:, :, :], in_=qkT_sp[:, :, 0:256])



_PROG = None


def _get_prog():
    global _PROG
    if _PROG is None:
        _PROG = _build()
    return _PROG


_RUN = None


def _get_runner():
    """Cached jitted SPMD executor (adapted from bass2jax.run_bass_via_pjrt)
    so repeated calls reuse the compiled NEFF for steady-state timing."""
    global _RUN
    if _RUN is not None:
        return _RUN
    import jax
    from jax.experimental.shard_map import shard_map
    from jax.sharding import Mesh, PartitionSpec
    from concourse import bass2jax

    nc = _get_prog()
    bass2jax.install_neuronx_cc_hook()
    partition_name = (nc.partition_id_tensor.name
                      if nc.partition_id_tensor else None)
    in_names, out_names, out_avals, zero_outs = [], [], [], []
    in_avals = []
    for alloc in nc.m.functions[0].allocations:
        if not isinstance(alloc, mybir.MemoryLocationSet):
            continue
        name = alloc.memorylocations[0].name
        if alloc.kind == "ExternalInput":
            if name != partition_name:
                in_names.append(name)
                in_avals.append(jax.core.ShapedArray(
                    tuple(alloc.tensor_shape), mybir.dt.np(alloc.dtype)))
        elif alloc.kind == "ExternalOutput":
            shape = tuple(alloc.tensor_shape)
            dtype = mybir.dt.np(alloc.dtype)
            out_names.append(name)
            out_avals.append(jax.core.ShapedArray(shape, dtype))
            zero_outs.append(np.zeros(shape, dtype))
    n_params = len(in_names)
    n_outs = len(out_avals)
    in_names = in_names + out_names
    if partition_name is not None:
        in_names.append(partition_name)
    donate = tuple(range(n_params, n_params + n_outs))

    def _body(*args):
        operands = list(args)
        if partition_name is not None:
            operands.append(bass2jax.partition_id_tensor())
        outs = bass2jax._bass_exec_p.bind(
            *operands,
            out_avals=tuple(out_avals),
            in_names=tuple(in_names),
            out_names=tuple(out_names),
            lowering_input_output_aliases=(),
            sim_require_finite=True,
            sim_require_nnan=True,
            nc=nc,
        )
        return tuple(outs)

    devices = jax.devices()[:NCORES]
    mesh = Mesh(np.asarray(devices), ("core",))
    in_specs = (PartitionSpec("core"),) * (n_params + n_outs)
    out_specs = (PartitionSpec("core"),) * n_outs

    global_avals = [
        jax.ShapeDtypeStruct((NCORES * a.shape[0], *a.shape[1:]), a.dtype)
        for a in in_avals + out_avals]

    def _compile_fn():
        jitted = jax.jit(
            shard_map(_body, mesh=mesh, in_specs=in_specs,
                      out_specs=out_specs, check_rep=False),
            donate_argnums=donate, keep_unused=True)
        return jitted.lower(*global_avals).compile()

    try:
        sharded = bass2jax.fast_dispatch_compile(_compile_fn)
    except Exception:
        sharded = jax.jit(
            shard_map(_body, mesh=mesh, in_specs=in_specs,
                      out_specs=out_specs, check_rep=False),
            donate_argnums=donate, keep_unused=True)
    _RUN = dict(fn=sharded, in_names=in_names, out_names=out_names,
                out_avals=out_avals, zero_outs=zero_outs, n_params=n_params,
                mesh=mesh)
    return _RUN


PIPE_N = 500


def _run_spmd(maps, time_iters=0):
    import jax
    from jax.sharding import NamedSharding, PartitionSpec
    import time as _time
    r = _get_runner()
    names = r["in_names"][:r["n_params"]]
    concat_in = [np.concatenate([np.asarray(maps[c][nm]) for c in
                                 range(NCORES)], axis=0) for nm in names]
    sh = NamedSharding(r["mesh"], PartitionSpec("core"))
    dev_in = [jax.device_put(a, sh) for a in concat_in]
    for a in dev_in:
        a.block_until_ready()

    zeros = [np.zeros((NCORES * z.shape[0], *z.shape[1:]), z.dtype)
             for z in r["zero_outs"]]
    # The kernel fully overwrites every ExternalOutput element, so each
    # timed call donates the previous call's output buffers: the chain
    # serializes executions on-device while the host streams dispatches.
    outs = r["fn"](*dev_in, *zeros)
    jax.block_until_ready(outs)
    times = []
    if time_iters:
        for _ in range(5):
            outs = r["fn"](*dev_in, *outs)
        jax.block_until_ready(outs)
        for _ in range(time_iters):
            t0 = _time.perf_counter()
            for _ in range(PIPE_N):
                outs = r["fn"](*dev_in, *outs)
            jax.block_until_ready(outs)
            times.append((_time.perf_counter() - t0) / PIPE_N)
    host = [np.asarray(a) for a in outs]
    res = [{nm: host[i].reshape(NCORES, *r["out_avals"][i].shape)[c]
            for i, nm in enumerate(r["out_names"])}
           for c in range(NCORES)]
    return res, times


def _shards(inputs):
    f = lambda x: np.ascontiguousarray(np.asarray(x), dtype=np.float32)
    hs2 = f(inputs["hidden_states"]).reshape(S, D)
    temb = f(inputs["temb"]).reshape(D)
    pi = np.concatenate([np.arange(0, HD, 2), np.arange(1, HD, 2)])
    cosp = f(np.asarray(inputs["rope_cos"])[:, pi].T)
    sinp = f(np.asarray(inputs["rope_sin"])[:, pi].T)
    sinp[0:64, :] *= -1.0
    q_w = f(inputs["q_w"]).reshape(HEADS, HD, D)[:, pi, :]
    k_w = f(inputs["k_w"]).reshape(HEADS, HD, D)[:, pi, :]
    v_w = f(inputs["v_w"])
    q_b = f(inputs["q_b"]).reshape(HEADS, HD)[:, pi]
    k_b = f(inputs["k_b"]).reshape(HEADS, HD)[:, pi]
    v_b = f(inputs["v_b"])
    mlp_w, mlp_b = f(inputs["mlp_w"]), f(inputs["mlp_b"])
    out_w, out_b = f(inputs["out_w"]), f(inputs["out_b"])
    norm_w, norm_b = f(inputs["norm_w"]), f(inputs["norm_b"])
    rmsq, rmsk = f(inputs["rms_q_w"])[pi], f(inputs["rms_k_w"])[pi]
    ident = np.eye(128, dtype=np.float32)

    maps = []
    for c in range(NCORES):
        hsl = slice(c * HPC, (c + 1) * HPC)
        vsl = slice(c * QKV, (c + 1) * QKV)
        msl = slice(c * MHC, (c + 1) * MHC)
        esl = slice(c * EMBC, (c + 1) * EMBC)
        qkvwT = np.ascontiguousarray(np.concatenate([
            q_w[hsl].reshape(QKV, D).T,
            k_w[hsl].reshape(QKV, D).T,
            v_w[vsl].T], axis=1)).astype(ml_dtypes.bfloat16)
        qkvb = np.concatenate([q_b[hsl].ravel(), k_b[hsl].ravel(), v_b[vsl]])
        outwT = np.ascontiguousarray(np.concatenate([
            out_w[:, vsl].T,
            out_w[:, D + c * MHC:D + (c + 1) * MHC].T], axis=0)).astype(
                ml_dtypes.bfloat16)
        maps.append({
            "hs": hs2,
            "hs_res": np.ascontiguousarray(hs2[c * SO:(c + 1) * SO]),
            "temb": temb, "cosT": cosp, "sinT": sinp,
            "qkvwT": qkvwT, "qkvb": np.ascontiguousarray(qkvb),
            "mlpwT": np.ascontiguousarray(mlp_w[msl].T).astype(
                ml_dtypes.bfloat16),
            "mlpb": np.ascontiguousarray(mlp_b[msl]),
            "outwT": outwT, "outb": out_b,
            "nwT": np.ascontiguousarray(norm_w[esl].T).astype(
                ml_dtypes.bfloat16),
            "nb": np.ascontiguousarray(norm_b[esl]),
            "rmsq": np.ascontiguousarray(rmsq),
            "rmsk": np.ascontiguousarray(rmsk),
            "ident": ident,
        })
    return maps


def kernel(**inputs):
    maps = _shards(inputs)
    res, times = _run_spmd(maps, time_iters=TIME_ITERS)
    LAST["results"] = res
    LAST["times"] = times
    out = np.concatenate([res[c]["out"] for c in range(NCORES)], axis=0)
    return out.reshape(1, S, D)



# revision 34
# speedup vs baseline: 1.5128x; 1.0814x over previous
"""Trainium2 Bass kernel for BriaFibo single transformer block.

Tensor-parallel over 8 NeuronCores: heads (24 -> 3/core) and mlp_hidden
(12288 -> 1536/core) are column-sharded; out projection row-sharded with a
device-side ReduceScatter.  AdaLN emb matvec is row-sharded + AllGather.
All big matmuls run in float32r (full PE rate at N>=256, ~fp32 storage).
"""

import ml_dtypes
import numpy as np

import concourse.bass as bass
import concourse.mybir as mybir
import concourse.tile as tile
from concourse import bacc
from concourse.bass_utils import run_bass_kernel_spmd

F32 = mybir.dt.float32
F32R = mybir.dt.float32r
BF16 = mybir.dt.bfloat16
AOP = mybir.AluOpType
AF = mybir.ActivationFunctionType

S, D = 2048, 3072
HEADS, HD = 24, 128
MH = 12288
NCORES = 8
HPC = HEADS // NCORES          # 3 heads/core
QKV = HPC * HD                 # 384
MHC = MH // NCORES             # 1536
CAT = QKV + MHC                # 1920
SO = S // NCORES               # 256 output rows/core
KT = D // 128                  # 24 contraction tiles
EMBC = 3 * D // NCORES         # 1152 adaLN rows/core
EPS_LN = 1e-6
EPS_RMS = 1e-6

TRACE = False
RSCHUNKS = 6
TIME_ITERS = 0
DEBUG = False
SIM = False
LAST = {}


def _r(ap):
    return ap.bitcast(F32R)



def _build():
    nc = bacc.Bacc("TRN2", target_bir_lowering=False, debug=False,
                   num_devices=NCORES)

    din = {}
    for name, shape, dt in [
        ("hs", [S, D], F32), ("hs_res", [SO, D], F32), ("temb", [D], F32),
        ("cosT", [HD, S], F32), ("sinT", [HD, S], F32),
        ("qkvwT", [D, 3 * QKV], BF16), ("qkvb", [3 * QKV], F32),
        ("mlpwT", [D, MHC], BF16), ("mlpb", [MHC], F32),
        ("outwT", [CAT, D], BF16), ("outb", [D], F32),
        ("nwT", [D, EMBC], BF16), ("nb", [EMBC], F32),
        ("rmsq", [HD], F32), ("rmsk", [HD], F32), ("ident", [128, 128], F32),
    ]:
        din[name] = nc.dram_tensor(name, shape, dt, kind="ExternalInput")
    out_d = nc.dram_tensor("out", [SO, D], F32, kind="ExternalOutput")
    dbg = {}
    if DEBUG:
        for name, shape in [("demb", [3 * D]), ("dnh", [KT, 128, 256]),
                            ("dqk", [2 * HPC, 128, 256]),
                            ("dv", [128, S // 128, QKV]),
                            ("dattn", [128, HPC, S]),
                            ("dhid", [128, 12, 256])]:
            dbg[name] = nc.dram_tensor(name, shape, F32,
                                       kind="ExternalOutput")

    from contextlib import ExitStack
    with tile.TileContext(nc) as tc, ExitStack() as ctx:
        _emit(ctx, nc, tc, din, out_d, dbg)
    nc.compile()
    return nc


def _emit(ctx, nc, tc, din, out_d, dbg=None):
    hs, hs_res = din["hs"], din["hs_res"]

    cpool = ctx.enter_context(tc.tile_pool(name="consts", bufs=1))
    dram = ctx.enter_context(tc.tile_pool(name="dram", bufs=1, space="DRAM"))

    ident_sb = cpool.tile([128, 128], F32)
    nc.sync.dma_start(out=ident_sb[:], in_=din["ident"][:, :])
    ident_bf = cpool.tile([128, 128], BF16)
    nc.gpsimd.dma_start(out=ident_bf[:], in_=din["ident"][:, :])
    ones_f = cpool.tile([128, 128], F32)
    nc.vector.memset(ones_f[:], 1.0)
    ones_col = cpool.tile([128, 1], F32R)         # lhsT for colsum -> [1,N]
    nc.vector.tensor_copy(ones_col[:], ones_f[:, 0:1])
    ones_col_bf = cpool.tile([128, 1], BF16)      # bf16 colsum lhsT
    nc.vector.tensor_copy(ones_col_bf[:], ones_f[:, 0:1])
    ones_row = cpool.tile([1, 128], F32)          # lhsT for bcast -> [128,N]
    nc.vector.tensor_copy(ones_row[:], ones_f[0:1, :])
    eps_ln_c = cpool.tile([128, 1], F32)
    nc.vector.memset(eps_ln_c[:], EPS_LN)
    eps_rms_c = cpool.tile([1, 1], F32)
    nc.vector.memset(eps_rms_c[:], EPS_RMS)

    rmsq_col = cpool.tile([128, 1], F32)
    nc.gpsimd.dma_start(out=rmsq_col[:],
                        in_=din["rmsq"].rearrange("(p one) -> p one", one=1))
    rmsk_col = cpool.tile([128, 1], F32)
    nc.gpsimd.dma_start(out=rmsk_col[:],
                        in_=din["rmsk"].rearrange("(p one) -> p one", one=1))
    qkvb_cols = cpool.tile([128, 9], F32)
    nc.gpsimd.dma_start(out=qkvb_cols[:],
                        in_=din["qkvb"].rearrange("(m p) -> p m", p=128))
    vb_b = cpool.tile([128, QKV], F32)
    vb_src = din["qkvb"][768:1152]
    nc.gpsimd.dma_start(
        out=vb_b[:],
        in_=bass.AP(vb_src.tensor, vb_src.offset, [[0, 128], [1, QKV]]))
    mlpb_cols = cpool.tile([128, 12], F32)
    nc.gpsimd.dma_start(out=mlpb_cols[:],
                        in_=din["mlpb"].rearrange("(m p) -> p m", p=128))

    # DRAM scratch
    nhT_sp = dram.tile([KT, 128, S], BF16)
    qkT_sp = dram.tile([2 * HPC, 128, S], BF16)
    ag_in = dram.tile([EMBC], F32)
    rk_b = dram.tile([S], F32)
    emb_all = dram.tile([3 * D], F32, addr_space="Shared")
    NC6 = 6                                     # out-proj column chunks
    CW = D // NC6                               # 512 cols per chunk
    RSCH = globals().get("RSCHUNKS", RSCHUNKS)  # collectives count
    RW = D // RSCH
    GPC = NC6 // RSCH                           # compute chunks per RS
    partial_c = [dram.tile([S, RW], BF16, name=f"partial{i}")
                 for i in range(RSCH)]
    rs_c = [dram.tile([SO, RW], BF16, name=f"rsc{i}") for i in range(RSCH)]

    # ---------------- Phase 0: AdaLN emb (sharded matvec + AllGather) ----
    with tc.tile_pool(name="p0", bufs=1) as p0, \
         tc.tile_pool(name="p0st", bufs=3) as p0st, \
         tc.tile_pool(name="p0ps", bufs=1, space="PSUM") as p0ps:
        temb_sb = p0.tile([128, KT], F32)
        nc.gpsimd.dma_start(out=temb_sb[:],
                            in_=din["temb"].rearrange("(a p) -> p a", p=128))
        silu_t = p0.tile([128, KT], BF16)
        nc.scalar.activation(silu_t[:], temb_sb[:], AF.Silu)
        pe_all = p0ps.tile([1, 3, 512], F32)
        for k in range(KT):
            nw_k = p0st.tile([128, EMBC], BF16, name="nw_k")
            nc.sync.dma_start(out=nw_k[:],
                              in_=din["nwT"][k * 128:(k + 1) * 128, :])
            for n in range(3):
                nc.tensor.matmul(pe_all[:, n, 0:384],
                                 silu_t[:, k:k + 1],
                                 nw_k[:, n * 384:(n + 1) * 384],
                                 start=(k == 0), stop=(k == KT - 1))
        nb_sb = p0.tile([1, EMBC], F32)
        nc.sync.dma_start(out=nb_sb[:],
                          in_=din["nb"].rearrange("(one a) -> one a", one=1))
        emb_row = p0.tile([1, EMBC], F32)
        for n in range(3):
            nc.vector.tensor_add(emb_row[:, n * 384:(n + 1) * 384],
                                 pe_all[:, n, 0:384],
                                 nb_sb[:, n * 384:(n + 1) * 384])
        nc.sync.dma_start(out=ag_in[:], in_=emb_row[:])
        if SIM:
            nc.sync.dma_start(out=emb_all[0:EMBC], in_=ag_in[:])
        else:
            nc.gpsimd.collective_compute(
                "AllGather", AOP.bypass,
                replica_groups=[list(range(NCORES))],
                ins=[ag_in.opt()], outs=[emb_all.opt()])

    if dbg:
        nc.sync.dma_start(out=dbg["demb"][:], in_=emb_all[:])

    scale_cols = cpool.tile([128, KT], F32)
    sc_src = emb_all[D:2 * D]
    nc.gpsimd.dma_start(
        out=scale_cols[:],
        in_=bass.AP(sc_src.tensor, sc_src.offset, [[1, 128], [128, KT]]))
    nc.vector.tensor_scalar_add(scale_cols[:], scale_cols[:], 1.0)
    shift_cols = cpool.tile([128, KT], F32)
    sh_src = emb_all[0:D]
    nc.gpsimd.dma_start(
        out=shift_cols[:],
        in_=bass.AP(sh_src.tensor, sh_src.offset, [[1, 128], [128, KT]]))

    # V stays SBUF-resident (bf16) from phase 1 through attention
    vres = ctx.enter_context(tc.tile_pool(name="vres", bufs=1))
    v_sb2 = vres.tile([128, S // 128, QKV], BF16)

    # ---------------- Phase 1: LN + transpose + qkv/v projections --------
    # bf16 pipeline, 512-token blocks, resident bf16 qkv weights
    NB2, BT2 = 4, 512
    with tc.tile_pool(name="p1w", bufs=1) as p1w, \
         tc.tile_pool(name="p1hs", bufs=2) as p1hs, \
         tc.tile_pool(name="p1st", bufs=3) as p1st, \
         tc.tile_pool(name="p1nh", bufs=2) as p1nh, \
         tc.tile_pool(name="p1ev", bufs=3) as p1ev, \
         tc.tile_pool(name="p1ps", bufs=2, space="PSUM") as p1ps, \
         tc.tile_pool(name="p1psT", bufs=2, space="PSUM") as p1psT:
        qkvw_k = [p1w.tile([128, 3 * QKV], BF16, name=f"qw{k}")
                  for k in range(KT)]
        for k in range(KT):
            nc.sync.dma_start(out=qkvw_k[k][:],
                              in_=din["qkvwT"][k * 128:(k + 1) * 128, :])
        for b in range(NB2):
            nhT_b = p1nh.tile([128, KT, BT2], BF16, name="nhT_b")
            hts = []
            for tt in range(4):
                row = b * BT2 + tt * 128
                h0 = p1hs.tile([128, D // 2], BF16, name=f"h{tt}a",
                               tag=f"h{tt}a")
                nc.gpsimd.dma_start(out=h0[:],
                                    in_=hs[row:row + 128, 0:D // 2])
                h1 = p1hs.tile([128, D // 2], BF16, name=f"h{tt}b",
                               tag=f"h{tt}b")
                nc.gpsimd.dma_start(out=h1[:],
                                    in_=hs[row:row + 128, D // 2:D])
                stats = p1st.tile([128, 6, 6], F32, name="stats")
                for g in range(3):
                    nc.vector.bn_stats(stats[:, g, :],
                                       h0[:, g * 512:(g + 1) * 512])
                    nc.vector.bn_stats(stats[:, 3 + g, :],
                                       h1[:, g * 512:(g + 1) * 512])
                mv = p1st.tile([128, 2], F32, name="mv")
                nc.vector.bn_aggr(mv[:], stats[:])
                sd = p1st.tile([128, 1], F32, name="sd")
                nc.scalar.activation(sd[:], mv[:, 1:2], AF.Sqrt,
                                     bias=eps_ln_c[:], scale=1.0)
                rstd = p1st.tile([128, 1], F32, name="rstd")
                nc.vector.reciprocal(rstd[:], sd[:])
                nc.vector.tensor_scalar(h0[:], h0[:], mv[:, 0:1], rstd[:],
                                        op0=AOP.subtract, op1=AOP.mult)
                nc.vector.tensor_scalar(h1[:], h1[:], mv[:, 0:1], rstd[:],
                                        op0=AOP.subtract, op1=AOP.mult)
                hts.append((h0, h1))
            for j in range(KT):
                psT = p1psT.tile([128, BT2], BF16, name="psT")
                for tt in range(4):
                    h0, h1 = hts[tt]
                    src = (h0[:, j * 128:(j + 1) * 128] if j < 12 else
                           h1[:, (j - 12) * 128:(j - 11) * 128])
                    nc.tensor.transpose(psT[:, tt * 128:(tt + 1) * 128],
                                        src, ident_bf[:])
                nc.vector.tensor_scalar(nhT_b[:, j, :], psT[:],
                                        scale_cols[:, j:j + 1],
                                        shift_cols[:, j:j + 1],
                                        op0=AOP.mult, op1=AOP.add)
                nc.gpsimd.dma_start(out=nhT_sp[j, :, b * BT2:(b + 1) * BT2],
                                    in_=nhT_b[:, j, :])
            for grp in range(2):          # q features then k features
                psqk = p1ps.tile([128, 3, BT2], F32, name="psqk", tag="pacc")
                for k in range(KT):
                    st, sp = (k == 0), (k == KT - 1)
                    for m in range(3):
                        mm = grp * 3 + m
                        nc.tensor.matmul(psqk[:, m, :],
                                         qkvw_k[k][:, mm * 128:(mm + 1) * 128],
                                         nhT_b[:, k, :], start=st, stop=sp)
                for m in range(3):
                    mm = grp * 3 + m
                    qks = p1ev.tile([128, BT2], BF16, name="qks")
                    nc.vector.tensor_scalar_add(qks[:], psqk[:, m, :],
                                                qkvb_cols[:, mm:mm + 1])
                    nc.sync.dma_start(
                        out=qkT_sp[mm, :, b * BT2:(b + 1) * BT2], in_=qks[:])
            for vh in range(2):           # bank-aligned [2,512] psum groups
                psv = p1ps.tile([128, 2, 512], F32, name="psv", tag="pacc")
                for k in range(KT):
                    st, sp = (k == 0), (k == KT - 1)
                    for mt2 in range(2):
                        mt = vh * 2 + mt2
                        nc.tensor.matmul(psv[:, mt2, 0:QKV],
                                         nhT_b[:, k, mt * 128:(mt + 1) * 128],
                                         qkvw_k[k][:, 768:1152],
                                         start=st, stop=sp)
                for mt2 in range(2):
                    nc.vector.tensor_add(v_sb2[:, b * 4 + vh * 2 + mt2, :],
                                         psv[:, mt2, 0:QKV], vb_b[:])

    # ---------------- Phase 2+3 shared: attnT accumulator ----------------
    with tc.tile_pool(name="attnp", bufs=1) as attnp:
        attnT = attnp.tile([128, HPC, S], BF16)

        # ------------- Phase 2: attention per head (software-pipelined) --
        with tc.tile_pool(name="p2cs", bufs=1) as p2cs, \
             tc.tile_pool(name="p2io", bufs=2) as p2io, \
             tc.tile_pool(name="p2big", bufs=1) as p2big, \
             tc.tile_pool(name="p2sc", bufs=2) as p2sc, \
             tc.tile_pool(name="p2sm", bufs=2) as p2sm, \
             tc.tile_pool(name="p2ex", bufs=2) as p2ex, \
             tc.tile_pool(name="p2ps_s", bufs=3, space="PSUM") as p2ps_s, \
             tc.tile_pool(name="p2ps_a", bufs=2, space="PSUM") as p2ps_a, \
             tc.tile_pool(name="p2ps_m", bufs=3, space="PSUM") as p2ps_m:
            cos_sb = p2cs.tile([128, S], BF16)
            nc.gpsimd.dma_start(out=cos_sb[:], in_=din["cosT"][:, :])
            sin_sb = p2cs.tile([128, S], BF16)
            nc.gpsimd.dma_start(out=sin_sb[:], in_=din["sinT"][:, :])

            def prologue(h):
                qT = p2io.tile([128, S], BF16, name="qT")
                nc.sync.dma_start(out=qT[:], in_=qkT_sp[h, :, :])
                kTt = p2io.tile([128, S], BF16, name="kTt")
                nc.sync.dma_start(out=kTt[:], in_=qkT_sp[HPC + h, :, :])

                rows_r = {}
                for nm, tsrc in (("q", qT), ("k", kTt)):
                    sq = p2big.tile([128, S], F32R, name="sq", tag="sqt")
                    nc.scalar.activation(sq[:], tsrc[:], AF.Square)
                    sd_row = p2sc.tile([1, S], F32, name="sd_row",
                                       tag="sd_row")
                    for n4 in range(4):
                        ms = p2ps_m.tile([1, 512], F32, name="ms",
                                         tag="pmisc")
                        nc.tensor.matmul(ms[:], ones_col[:],
                                         sq[:, n4 * 512:(n4 + 1) * 512],
                                         start=True, stop=True)
                        nc.scalar.activation(
                            sd_row[:, n4 * 512:(n4 + 1) * 512],
                            ms[:], AF.Sqrt, bias=eps_rms_c[:],
                            scale=1.0 / HD)
                    rrow = p2sc.tile([1, S], F32, name="rrow_" + nm,
                                     tag="rrow" + nm)
                    nc.vector.reciprocal(rrow[:], sd_row[:])
                    rows_r[nm] = rrow
                # rstd_k columns via DRAM bounce (hidden by pipelining)
                nc.sync.dma_start(out=rk_b[:], in_=rows_r["k"][:])
                rstdk_cols = p2sc.tile([128, 16], F32, name="rstdk_cols")
                nc.gpsimd.dma_start(
                    out=rstdk_cols[:],
                    in_=rk_b.rearrange("(a p) -> p a", p=128))
                nc.vector.tensor_scalar_mul(rstdk_cols[:], rstdk_cols[:],
                                            1.0 / float(np.sqrt(HD)))

                nc.vector.tensor_scalar_mul(qT[:], qT[:], rmsq_col[:])
                nc.vector.tensor_scalar_mul(kTt[:], kTt[:], rmsk_col[:])

                # q *= rstd_q (rank-1 PE broadcast; commutes with rope)
                for n4 in range(4):
                    n4s = slice(n4 * 512, (n4 + 1) * 512)
                    bq = p2ps_m.tile([128, 512], F32, name="bq", tag="pmisc")
                    nc.tensor.matmul(bq[:], ones_row[:],
                                     rows_r["q"][:, n4s],
                                     start=True, stop=True)
                    nc.vector.tensor_mul(qT[:, n4s], qT[:, n4s], bq[:])

                # rope: out = x*cos + swap(x)*sin_signed
                def rope_sum(dst, srct):
                    sw = p2big.tile([128, S], BF16, name="ropesw",
                                    tag="ropesw")
                    nc.gpsimd.dma_start(out=sw[0:64, :], in_=srct[64:128, :])
                    nc.gpsimd.dma_start(out=sw[64:128, :], in_=srct[0:64, :])
                    t1 = p2big.tile([128, S], BF16, name="ropet1",
                                    tag="ropet1")
                    nc.vector.tensor_mul(t1[:], srct[:], cos_sb[:])
                    nc.vector.tensor_mul(sw[:], sw[:], sin_sb[:])
                    nc.vector.tensor_add(dst[:], t1[:], sw[:])

                qr = p2sc.tile([128, S], BF16, name="qr")
                rope_sum(qr, qT)
                kr = p2sc.tile([128, S], BF16, name="kr")
                rope_sum(kr, kTt)
                return dict(qr=qr, kr=kr, rstdk=rstdk_cols)

            def qc_loop(h, pro):
                qr, kr, rstdk_cols = pro["qr"], pro["kr"], pro["rstdk"]
                for qc in range(4):
                    qsl = slice(qc * 512, (qc + 1) * 512)
                    expS = p2ex.tile([128, 16, 512], BF16, name="expS")
                    for kk in range(16):
                        ps_s = p2ps_s.tile([128, 512], F32, name="ps_s")
                        nc.tensor.matmul(ps_s[:],
                                         kr[:, kk * 128:(kk + 1) * 128],
                                         qr[:, qsl], start=True, stop=True)
                        nc.scalar.activation(expS[:, kk, :], ps_s[:], AF.Exp,
                                             scale=rstdk_cols[:, kk:kk + 1])
                    dtr = p2sm.tile([128, 8, 512], BF16, name="dtr",
                                    tag="dtr")
                    for i in range(8):
                        nc.vector.tensor_add(dtr[:, i, :], expS[:, 2 * i, :],
                                             expS[:, 2 * i + 1, :])
                    for i in range(4):
                        nc.vector.tensor_add(dtr[:, i, :], dtr[:, 2 * i, :],
                                             dtr[:, 2 * i + 1, :])
                    for i in range(2):
                        nc.vector.tensor_add(dtr[:, i, :], dtr[:, 2 * i, :],
                                             dtr[:, 2 * i + 1, :])
                    nc.vector.tensor_add(dtr[:, 0, :], dtr[:, 0, :],
                                         dtr[:, 1, :])
                    ps_d = p2ps_m.tile([1, 512], F32, name="ps_d",
                                       tag="pmisc")
                    nc.tensor.matmul(ps_d[:], ones_col_bf[:], dtr[:, 0, :],
                                     start=True, stop=True)
                    rec_row = p2sm.tile([1, 512], F32, name="rec_row")
                    nc.vector.reciprocal(rec_row[:], ps_d[:])
                    ps_db = p2ps_m.tile([128, 512], F32, name="ps_db",
                                        tag="pmisc")
                    nc.tensor.matmul(ps_db[:], ones_row[:], rec_row[:],
                                     start=True, stop=True)
                    den_sb = p2sm.tile([128, 512], F32, name="den_sb")
                    nc.vector.tensor_copy(den_sb[:], ps_db[:])
                    ps_a = p2ps_a.tile([128, 512], F32, name="ps_a")
                    for kk in range(16):
                        nc.tensor.matmul(
                            ps_a[:], v_sb2[:, kk, h * 128:(h + 1) * 128],
                            expS[:, kk, :],
                            start=(kk == 0), stop=(kk == 15))
                    nc.vector.tensor_mul(attnT[:, h, qsl], ps_a[:],
                                         den_sb[:])

            pros = []
            for h in range(HPC):
                pros.append(prologue(h))
                if h > 0:
                    qc_loop(h - 1, pros[h - 1])
            qc_loop(HPC - 1, pros[HPC - 1])

        # ------------- Phase 3a: MLP hidden (resident weights) -----------
        with tc.tile_pool(name="p3hid", bufs=1) as p3hid:
            hidT = p3hid.tile([128, 12, S], BF16, name="hidT")
            with tc.tile_pool(name="p3nh", bufs=2) as p3nh, \
                 tc.tile_pool(name="p3mw", bufs=1) as p3mw, \
                 tc.tile_pool(name="p3psh", bufs=7,
                              space="PSUM") as p3psh:
                mwk = [p3mw.tile([128, MHC], BF16, name=f"mw{k}")
                       for k in range(KT)]
                for k in range(KT):
                    nc.sync.dma_start(
                        out=mwk[k][:],
                        in_=din["mlpwT"][k * 128:(k + 1) * 128, :])
                for tc4 in range(4):
                    toff = tc4 * 512
                    nhT_c = p3nh.tile([128, KT, 512], BF16, name="nhT_c")
                    nc.gpsimd.dma_start(
                        out=nhT_c[:],
                        in_=nhT_sp[:, :, toff:toff + 512].rearrange(
                            "j p t -> p j t"))
                    for hh in range(2):
                        ps_hs = [p3psh.tile([128, 512], F32, name="ps_h",
                                            tag="psh") for _ in range(6)]
                        for k in range(KT):
                            for m in range(6):
                                nc.tensor.matmul(
                                    ps_hs[m][:],
                                    mwk[k][:, hh * 768 + m * 128:
                                           hh * 768 + (m + 1) * 128],
                                    nhT_c[:, k, :],
                                    start=(k == 0), stop=(k == KT - 1))
                        for m in range(6):
                            idx = hh * 6 + m
                            nc.scalar.activation(
                                hidT[:, idx, toff:toff + 512],
                                ps_hs[m][:], AF.Gelu_apprx_tanh,
                                bias=mlpb_cols[:, idx:idx + 1], scale=1.0)

            # --------- Phase 3b: out-projection + chunked ReduceScatter --
            with tc.tile_pool(name="p3ow", bufs=2) as p3ow, \
                 tc.tile_pool(name="p3ev", bufs=4) as p3ev, \
                 tc.tile_pool(name="p4", bufs=2) as p4, \
                 tc.tile_pool(name="p4c", bufs=1) as p4c, \
                 tc.tile_pool(name="p3pso", bufs=6, space="PSUM") as p3pso:
                gate_b = p4c.tile([128, D], F32)
                g_src = emb_all[2 * D:3 * D]
                nc.gpsimd.dma_start(
                    out=gate_b[:],
                    in_=bass.AP(g_src.tensor, g_src.offset,
                                [[0, 128], [1, D]]))
                outb_b = p4c.tile([128, D], F32)
                ob_src = din["outb"][0:D]
                nc.gpsimd.dma_start(
                    out=outb_b[:],
                    in_=bass.AP(ob_src.tensor, ob_src.offset,
                                [[0, 128], [1, D]]))
                NKO = CAT // 128
                for n6 in range(NC6):
                    ncol = slice(n6 * CW, (n6 + 1) * CW)
                    ow = p3ow.tile([128, NKO, CW], BF16, name="ow")
                    nc.sync.dma_start(
                        out=ow[:],
                        in_=din["outwT"].rearrange(
                            "(ko p) n -> p ko n", p=128)[:, :, ncol])
                    for m16 in range(16):
                        msl = slice(m16 * 128, (m16 + 1) * 128)
                        ps_o = p3pso.tile([128, CW], F32, name="ps_o",
                                          tag="pso")
                        for k in range(NKO):
                            lhsT = (attnT[:, k, msl] if k < HPC else
                                    hidT[:, k - HPC, msl])
                            nc.tensor.matmul(ps_o[:], lhsT, ow[:, k, :],
                                             start=(k == 0),
                                             stop=(k == NKO - 1))
                        po = p3ev.tile([128, CW], BF16, name="po")
                        nc.vector.tensor_copy(po[:], ps_o[:])
                        ri, rc = n6 // GPC, (n6 % GPC) * CW
                        nc.gpsimd.dma_start(
                            out=partial_c[ri][msl, rc:rc + CW], in_=po[:])
                    if n6 % GPC == GPC - 1:
                        ri = n6 // GPC
                        if SIM:
                            nc.sync.dma_start(out=rs_c[ri][:, :],
                                              in_=partial_c[ri][0:SO, :])
                        else:
                            nc.gpsimd.collective_compute(
                                "ReduceScatter", AOP.add,
                                replica_groups=[list(range(NCORES))],
                                ins=[partial_c[ri].opt()],
                                outs=[rs_c[ri].opt()])
                        rsl = slice(ri * RW, (ri + 1) * RW)
                        for t in range(2):
                            tsl = slice(t * 128, (t + 1) * 128)
                            rt = p4.tile([128, RW], BF16, name="rt")
                            nc.sync.dma_start(out=rt[:],
                                              in_=rs_c[ri][tsl, :])
                            ht = p4.tile([128, RW], F32, name="ht")
                            nc.scalar.dma_start(out=ht[:],
                                                in_=hs_res[tsl, rsl])
                            tmp = p4.tile([128, RW], F32, name="tmp")
                            nc.vector.tensor_add(tmp[:], rt[:],
                                                 outb_b[:, rsl])
                            nc.vector.tensor_mul(tmp[:], tmp[:],
                                                 gate_b[:, rsl])
                            nc.vector.tensor_add(tmp[:], tmp[:], ht[:])
                            nc.sync.dma_start(out=out_d[tsl, rsl],
                                              in_=tmp[:])
            if dbg:
                nc.gpsimd.dma_start(out=dbg["dv"][:, :, :], in_=v_sb2[:, :, :])
                nc.gpsimd.dma_start(out=dbg["dattn"][:, :, :],
                                    in_=attnT[:, :, :])
                nc.gpsimd.dma_start(out=dbg["dhid"][:, :, :],
                                    in_=hidT[:, :, 0:256])

    if dbg:
        nc.gpsimd.dma_start(out=dbg["dnh"][:, :, :], in_=nhT_sp[:, :, 0:256])
        nc.gpsimd.dma_start(out=dbg["dqk"][:, :, :], in_=qkT_sp[:, :, 0:256])



_PROG = None


def _get_prog():
    global _PROG
    if _PROG is None:
        _PROG = _build()
    return _PROG


_RUN = None


def _get_runner():
    """Cached jitted SPMD executor (adapted from bass2jax.run_bass_via_pjrt)
    so repeated calls reuse the compiled NEFF for steady-state timing."""
    global _RUN
    if _RUN is not None:
        return _RUN
    import jax
    from jax.experimental.shard_map import shard_map
    from jax.sharding import Mesh, PartitionSpec
    from concourse import bass2jax

    nc = _get_prog()
    bass2jax.install_neuronx_cc_hook()
    partition_name = (nc.partition_id_tensor.name
                      if nc.partition_id_tensor else None)
    in_names, out_names, out_avals, zero_outs = [], [], [], []
    in_avals = []
    for alloc in nc.m.functions[0].allocations:
        if not isinstance(alloc, mybir.MemoryLocationSet):
            continue
        name = alloc.memorylocations[0].name
        if alloc.kind == "ExternalInput":
            if name != partition_name:
                in_names.append(name)
                in_avals.append(jax.core.ShapedArray(
                    tuple(alloc.tensor_shape), mybir.dt.np(alloc.dtype)))
        elif alloc.kind == "ExternalOutput":
            shape = tuple(alloc.tensor_shape)
            dtype = mybir.dt.np(alloc.dtype)
            out_names.append(name)
            out_avals.append(jax.core.ShapedArray(shape, dtype))
            zero_outs.append(np.zeros(shape, dtype))
    n_params = len(in_names)
    n_outs = len(out_avals)
    in_names = in_names + out_names
    if partition_name is not None:
        in_names.append(partition_name)
    donate = tuple(range(n_params, n_params + n_outs))

    def _body(*args):
        operands = list(args)
        if partition_name is not None:
            operands.append(bass2jax.partition_id_tensor())
        outs = bass2jax._bass_exec_p.bind(
            *operands,
            out_avals=tuple(out_avals),
            in_names=tuple(in_names),
            out_names=tuple(out_names),
            lowering_input_output_aliases=(),
            sim_require_finite=True,
            sim_require_nnan=True,
            nc=nc,
        )
        return tuple(outs)

    devices = jax.devices()[:NCORES]
    mesh = Mesh(np.asarray(devices), ("core",))
    in_specs = (PartitionSpec("core"),) * (n_params + n_outs)
    out_specs = (PartitionSpec("core"),) * n_outs

    global_avals = [
        jax.ShapeDtypeStruct((NCORES * a.shape[0], *a.shape[1:]), a.dtype)
        for a in in_avals + out_avals]

    def _compile_fn():
        jitted = jax.jit(
            shard_map(_body, mesh=mesh, in_specs=in_specs,
                      out_specs=out_specs, check_rep=False),
            donate_argnums=donate, keep_unused=True)
        return jitted.lower(*global_avals).compile()

    try:
        sharded = bass2jax.fast_dispatch_compile(_compile_fn)
    except Exception:
        sharded = jax.jit(
            shard_map(_body, mesh=mesh, in_specs=in_specs,
                      out_specs=out_specs, check_rep=False),
            donate_argnums=donate, keep_unused=True)
    _RUN = dict(fn=sharded, in_names=in_names, out_names=out_names,
                out_avals=out_avals, zero_outs=zero_outs, n_params=n_params,
                mesh=mesh)
    return _RUN


PIPE_N = 1500


def _run_spmd(maps, time_iters=0):
    import jax
    from jax.sharding import NamedSharding, PartitionSpec
    import time as _time
    r = _get_runner()
    names = r["in_names"][:r["n_params"]]
    concat_in = [np.concatenate([np.asarray(maps[c][nm]) for c in
                                 range(NCORES)], axis=0) for nm in names]
    sh = NamedSharding(r["mesh"], PartitionSpec("core"))
    dev_in = [jax.device_put(a, sh) for a in concat_in]
    for a in dev_in:
        a.block_until_ready()

    zeros = [np.zeros((NCORES * z.shape[0], *z.shape[1:]), z.dtype)
             for z in r["zero_outs"]]
    # The kernel fully overwrites every ExternalOutput element, so each
    # timed call donates the previous call's output buffers: the chain
    # serializes executions on-device while the host streams dispatches.
    outs = r["fn"](*dev_in, *zeros)
    jax.block_until_ready(outs)
    times = []
    if time_iters:
        for _ in range(5):
            outs = r["fn"](*dev_in, *outs)
        jax.block_until_ready(outs)
        for _ in range(time_iters):
            t0 = _time.perf_counter()
            for _ in range(PIPE_N):
                outs = r["fn"](*dev_in, *outs)
            jax.block_until_ready(outs)
            times.append((_time.perf_counter() - t0) / PIPE_N)
    host = [np.asarray(a) for a in outs]
    res = [{nm: host[i].reshape(NCORES, *r["out_avals"][i].shape)[c]
            for i, nm in enumerate(r["out_names"])}
           for c in range(NCORES)]
    return res, times


def _shards(inputs):
    f = lambda x: np.ascontiguousarray(np.asarray(x), dtype=np.float32)
    hs2 = f(inputs["hidden_states"]).reshape(S, D)
    temb = f(inputs["temb"]).reshape(D)
    pi = np.concatenate([np.arange(0, HD, 2), np.arange(1, HD, 2)])
    cosp = f(np.asarray(inputs["rope_cos"])[:, pi].T)
    sinp = f(np.asarray(inputs["rope_sin"])[:, pi].T)
    sinp[0:64, :] *= -1.0
    q_w = f(inputs["q_w"]).reshape(HEADS, HD, D)[:, pi, :]
    k_w = f(inputs["k_w"]).reshape(HEADS, HD, D)[:, pi, :]
    v_w = f(inputs["v_w"])
    q_b = f(inputs["q_b"]).reshape(HEADS, HD)[:, pi]
    k_b = f(inputs["k_b"]).reshape(HEADS, HD)[:, pi]
    v_b = f(inputs["v_b"])
    mlp_w, mlp_b = f(inputs["mlp_w"]), f(inputs["mlp_b"])
    out_w, out_b = f(inputs["out_w"]), f(inputs["out_b"])
    norm_w, norm_b = f(inputs["norm_w"]), f(inputs["norm_b"])
    rmsq, rmsk = f(inputs["rms_q_w"])[pi], f(inputs["rms_k_w"])[pi]
    ident = np.eye(128, dtype=np.float32)

    maps = []
    for c in range(NCORES):
        hsl = slice(c * HPC, (c + 1) * HPC)
        vsl = slice(c * QKV, (c + 1) * QKV)
        msl = slice(c * MHC, (c + 1) * MHC)
        esl = slice(c * EMBC, (c + 1) * EMBC)
        qkvwT = np.ascontiguousarray(np.concatenate([
            q_w[hsl].reshape(QKV, D).T,
            k_w[hsl].reshape(QKV, D).T,
            v_w[vsl].T], axis=1)).astype(ml_dtypes.bfloat16)
        qkvb = np.concatenate([q_b[hsl].ravel(), k_b[hsl].ravel(), v_b[vsl]])
        outwT = np.ascontiguousarray(np.concatenate([
            out_w[:, vsl].T,
            out_w[:, D + c * MHC:D + (c + 1) * MHC].T], axis=0)).astype(
                ml_dtypes.bfloat16)
        maps.append({
            "hs": hs2,
            "hs_res": np.ascontiguousarray(hs2[c * SO:(c + 1) * SO]),
            "temb": temb, "cosT": cosp, "sinT": sinp,
            "qkvwT": qkvwT, "qkvb": np.ascontiguousarray(qkvb),
            "mlpwT": np.ascontiguousarray(mlp_w[msl].T).astype(
                ml_dtypes.bfloat16),
            "mlpb": np.ascontiguousarray(mlp_b[msl]),
            "outwT": outwT, "outb": out_b,
            "nwT": np.ascontiguousarray(norm_w[esl].T).astype(
                ml_dtypes.bfloat16),
            "nb": np.ascontiguousarray(norm_b[esl]),
            "rmsq": np.ascontiguousarray(rmsq),
            "rmsk": np.ascontiguousarray(rmsk),
            "ident": ident,
        })
    return maps


def kernel(**inputs):
    maps = _shards(inputs)
    res, times = _run_spmd(maps, time_iters=TIME_ITERS)
    LAST["results"] = res
    LAST["times"] = times
    out = np.concatenate([res[c]["out"] for c in range(NCORES)], axis=0)
    return out.reshape(1, S, D)

